# revision 27
# baseline (speedup 1.0000x reference)
"""Trainium2 Bass kernel for AlphaFold-style gated MSA attention.

Reference computation (per batch b=1, per MSA row n of 64):
    q = (q_x @ wq) / sqrt(32);  k = k_x @ wk;  v = v_x @ wv      (heads: 8 x 32)
    a = softmax(q k^T + bias_mask[n,k] + bias_pair[h,q,k])
    o = (a @ v) * sigmoid(q_x @ wg + bg)
    out = o @ wo + bo

Distribution: data-parallel over the 64 MSA rows -> 8 rows per NeuronCore.

Per-core schedule (per row n):
  1. Projections in f32r from pre-transposed [C, seq] inputs; PSUM results
     are cast to bf16 on evacuation (qT/kT/v for the bf16 attention
     matmuls; the gate tanh comes out of ACT directly in bf16).
  2. S^T = k_h q_h^T per head-pair/key-chunk (bf16, K=32) using 2-way PE
     row tiling: the two matmuls of a pair target disjoint 32-row bands
     (tile_position) and distinct PSUM banks, so they stream concurrently.
     bias_mask folds into the ACT exp as a per-partition bias (S^T layout
     puts keys on partitions); softmax max-subtraction is skipped (logits
     are O(5), far from overflow). exp writes bf16.
  3. bias_pair is applied multiplicatively: the host ships exp(bias_pair)
     in bf16 and E *= expBP runs on DVE (2x bf16 mode) for 6 heads and
     GPSIMD for 2 heads (engine balance).
  4. o^T = V^T E per head with 4-way PE column tiling: head h2 writes PSUM
     partitions 32*h2 of ONE bank, so the group's output is born packed
     [4*32, q] - no banding DMAs. The softmax denominators come from a
     second column-tiled pass with ones weights (M=1 per head).
  5. Tail (software-pipelined one row late): reciprocal of denominators,
     broadcast via DRAM round-trip, gate * recip fold (GPSIMD), output
     gating mul (DVE, bf16 2x), then the output projection in bf16.
"""

import math
import os
import sys

for _p in ("/opt/trn_rl_repo", "/root/.axon_site/_ro/trn_rl_repo"):
    if os.path.isdir(_p) and _p not in sys.path:
        sys.path.append(_p)

import numpy as np

import bass_rust
import concourse.bass as bass
import concourse.mybir as mybir
import concourse.tile as tile
from concourse.bass_utils import run_bass_kernel_spmd
from concourse.masks import make_identity
from concourse.tile import ScopedClock

f32 = mybir.dt.float32
f32r = mybir.dt.float32r
bf16 = mybir.dt.bfloat16

N_CORES = 8
NL = 8        # MSA rows per core (64 / 8)
SEQ = 512     # q and k sequence length
C = 256       # channel dim of q_x/k_x/v_x and the output
HID = 256     # heads * c_hidden
H = 8         # heads
CH = 32       # c_hidden per head
P = 128
CC = C // P   # 2 contraction chunks for projections
HC = HID // P  # 2 hidden chunks
KC = SEQ // P  # 4 key chunks
QC = SEQ // P  # 4 query chunks
HG = 2        # head groups of 4

# Engine assignment for the bias_pair application (balance knobs):
# PE_PAIRS get an ADDITIVE bias via identity matmuls into the S PSUM
# (host ships raw bf16 bias_pair for those heads); the rest get the
# multiplicative exp(bias_pair) on DVE, except GPS_TRIPLES (hg, pr, kc)
# which run on GPSIMD.
PE_PAIRS = {(1, 1)}
GPS_TRIPLES = {(0, 1, 0), (0, 1, 1), (0, 1, 2), (0, 1, 3), (1, 0, 1)}


class _TileContextSplitWaits(tile.TileContext):
    """This container's walrus supports ONE sync-wait per instruction (the
    TRN2 EVENTS struct has a single wait slot and this build refuses to
    expand multi-wait instructions). Tile attaches several waits to one
    instruction; split the extras onto same-engine NOPs emitted just before
    it — the engine queue is in-order, so this is semantically identical."""

    def _add_instruction(self, inst):
        si = inst.sync_info
        if (
            si is not None
            and len(si.on_wait) > 1
            and inst.engine != mybir.EngineType.Unassigned
        ):
            waits = list(si.on_wait)
            for w in waits[:-1]:
                nop = mybir.InstNoOp(
                    name=self.nc.get_next_instruction_name(),
                    sync_info=mybir.SyncInfo(on_wait=[w], on_update=[]),
                    bass_nofuse=True,
                    engine=inst.engine,
                )
                super()._add_instruction(nop)
            inst.sync_info = mybir.SyncInfo(
                on_wait=waits[-1:], on_update=list(si.on_update)
            )
        super()._add_instruction(inst)

    def _drain_and_barrier(self, tick_clock, wait_clock):
        nc = self.nc
        drain_inst = nc.sync.drain()
        wait_clock.add_sem_waits(
            drain_inst.ins, ScopedClock({None: tick_clock.global_clock})
        )
        si = drain_inst.ins.sync_info
        if si is not None and len(si.on_wait) > 1:
            waits = list(si.on_wait)
            updates = list(si.on_update)
            drain_inst.ins.sync_info = bass_rust.SyncInfo(
                on_wait=waits[:1], on_update=[]
            )
            for i, w in enumerate(waits[1:]):
                upd = updates if i == len(waits) - 2 else []
                nop = nc.sync.nop()
                nop.ins.sync_info = bass_rust.SyncInfo(on_wait=[w], on_update=upd)
        nc.all_engine_barrier()
        assert self.sems is not None
        popped = nc._tile_sem_poison_stack.pop()
        assert popped is self._sem_poison
        nc.clear_and_free_semaphores(list(self.sems.allocated().values()))
        nc.all_engine_barrier()


def _build_nc():
    nc = bass.Bass(
        "TRN2", target_bir_lowering=False, debug=False, num_devices=N_CORES
    )
    qx = nc.dram_tensor("qx", [NL, C, SEQ], f32r, kind="ExternalInput").ap()
    kx = nc.dram_tensor("kx", [NL, C, SEQ], f32r, kind="ExternalInput").ap()
    vx = nc.dram_tensor("vx", [NL, C, SEQ], f32r, kind="ExternalInput").ap()
    # exp(bias_pair) transposed [h, k, q], bf16
    bpt = nc.dram_tensor("bpt", [H, SEQ, SEQ], bf16, kind="ExternalInput").ap()
    bm = nc.dram_tensor("bm", [P, KC, NL], f32, kind="ExternalInput").ap()
    wq = nc.dram_tensor("wq", [C, HID], f32r, kind="ExternalInput").ap()
    wk = nc.dram_tensor("wk", [C, HID], f32r, kind="ExternalInput").ap()
    wv = nc.dram_tensor("wv", [C, HID], f32r, kind="ExternalInput").ap()
    wg = nc.dram_tensor("wg", [C, HID], f32r, kind="ExternalInput").ap()
    bgh = nc.dram_tensor("bgh", [P, HC], f32, kind="ExternalInput").ap()
    wo = nc.dram_tensor("wo", [HID, C], bf16, kind="ExternalInput").ap()
    bo_bc = nc.dram_tensor("bo_bc", [P, C], f32, kind="ExternalInput").ap()
    out = nc.dram_tensor("out", [NL, SEQ, C], f32, kind="ExternalOutput").ap()
    dbg = {}
    _flags = set(
        f for f in os.environ.get("BASS_DEBUG_OUT", "").split(",") if f
    )
    if "all" in _flags:
        _flags = {"den", "ot", "rbc", "es", "otg", "osb"}
    _shapes = {
        "den": [NL, H, SEQ],
        "ot": [NL, P, HG, SEQ],
        "rbc": [NL, P, HG, SEQ],
        "es": [NL, P, 2, SEQ],
        "otg": [NL, P, HG, SEQ],
        "osb": [NL, P, C],
    }
    for f in _flags:
        dbg[f] = nc.dram_tensor(
            f + "_dbg", _shapes[f], f32, kind="ExternalOutput"
        ).ap()

    Exp = mybir.ActivationFunctionType.Exp
    Tanh = mybir.ActivationFunctionType.Tanh
    MULT = mybir.AluOpType.mult
    ADD = mybir.AluOpType.add

    with _TileContextSplitWaits(nc) as tc:
        with (
            tc.tile_pool(name="const", bufs=1) as const,
            tc.tile_pool(name="dram", bufs=2, space="DRAM") as drp,
        ):
            # --- constants ---------------------------------------------------
            w_sbs = {}
            for name, w_ap in (("wq", wq), ("wk", wk), ("wv", wv), ("wg", wg)):
                w_sbs[name] = const.tile(
                    [P, CC, HID], f32r, tag=f"w_{name}", name=f"w_{name}"
                )
                nc.sync.dma_start(
                    out=w_sbs[name],
                    in_=w_ap.rearrange("(cc p) h -> p cc h", p=P),
                )
            wo_sb = const.tile([P, HC, C], bf16, tag="w_wo")
            nc.sync.dma_start(
                out=wo_sb, in_=wo.rearrange("(hc p) c -> p hc c", p=P)
            )
            bm_sb = const.tile([P, KC, NL], f32, tag="bm")
            nc.sync.dma_start(out=bm_sb, in_=bm)
            bgh_sb = const.tile([P, HC], f32, tag="bgh")
            nc.sync.dma_start(out=bgh_sb, in_=bgh)
            bo_sb = const.tile([P, C], f32, tag="bo")
            nc.sync.dma_start(out=bo_sb, in_=bo_bc)
            ones_w = const.tile([P, 4], bf16, tag="ones_w")
            nc.vector.memset(ones_w, 1.0)
            ones_bc = const.tile([P, CH], bf16, tag="ones_bc")
            nc.vector.memset(ones_bc, 1.0)
            ident_f = const.tile([P, P], f32, tag="ident_f")
            make_identity(nc, ident_f)
            ident_b = const.tile([P, P], bf16, tag="ident_b")
            nc.vector.tensor_copy(ident_b, ident_f)

            # --- main loop ---------------------------------------------------
            with (
                tc.tile_pool(name="xt", bufs=2) as xt,
                tc.tile_pool(name="pj", bufs=1) as pj,
                tc.tile_pool(name="gh", bufs=2) as gh,
                tc.tile_pool(name="vv", bufs=2) as vv,
                tc.tile_pool(name="ee", bufs=4) as ee,
                tc.tile_pool(name="ot", bufs=2) as ot,
                tc.tile_pool(name="dn", bufs=2) as dn,
                tc.tile_pool(name="gp", bufs=2) as gp,
                tc.tile_pool(name="ou", bufs=2) as ou,
                tc.tile_pool(name="psQ", bufs=2, space="PSUM") as psQ,
                tc.tile_pool(name="psO", bufs=1, space="PSUM") as psO,
                tc.tile_pool(name="psD", bufs=1, space="PSUM") as psD,
                tc.tile_pool(name="psA", bufs=2, space="PSUM") as psA,
            ):
                def emit_xt(n):
                    # A: inputs arrive pre-transposed [C, seq], f32r in DRAM.
                    xTs = {}
                    for name, src_ap in (("q", qx), ("k", kx), ("v", vx)):
                        xT = xt.tile([P, CC, SEQ], f32r, tag=f"xt_{name}")
                        nc.sync.dma_start(
                            out=xT,
                            in_=src_ap[n].rearrange("(cc p) s -> p cc s", p=P),
                        )
                        xTs[name] = xT
                    return xTs

                # Row 0's input DMAs go out BEFORE the big bias_pair const
                # load: the SP queue is in-order, so row 0's projections
                # would otherwise wait for the full 4MB bpt transfer.
                xTs0 = emit_xt(0)
                bpt_sb = const.tile([P, H, KC, SEQ], bf16, tag="bpt")
                for h in range(H):
                    nc.sync.dma_start(
                        out=bpt_sb[:, h],
                        in_=bpt[h].rearrange("(kc p) q -> p kc q", p=P),
                    )

                def emit_front(n, xTs=None):
                    if xTs is None:
                        xTs = emit_xt(n)

                    # B: projections (f32r matmuls, bf16 evacuation)
                    qT = pj.tile([P, HC, SEQ], bf16, tag="qT")
                    kT = pj.tile([P, HC, SEQ], bf16, tag="kT")
                    for dst, wname, src in (
                        (qT, "wq", xTs["q"]),
                        (kT, "wk", xTs["k"]),
                    ):
                        for hc in range(HC):
                            pp = psA.tile([P, SEQ], f32, tag="psA")
                            for cc in range(CC):
                                nc.tensor.matmul(
                                    pp,
                                    w_sbs[wname][:, cc, P * hc : P * (hc + 1)],
                                    src[:, cc, :],
                                    start=(cc == 0),
                                    stop=(cc == CC - 1),
                                )
                            nc.vector.tensor_copy(dst[:, hc, :], pp)

                    gth = gh.tile([P, HC, SEQ], bf16, tag="gth")
                    for hc in range(HC):
                        pp = psA.tile([P, SEQ], f32, tag="psA")
                        for cc in range(CC):
                            nc.tensor.matmul(
                                pp,
                                w_sbs["wg"][:, cc, P * hc : P * (hc + 1)],
                                xTs["q"][:, cc, :],
                                start=(cc == 0),
                                stop=(cc == CC - 1),
                            )
                        # sigmoid(x + bg) = 0.5*tanh((x + bg)/2) + 0.5
                        nc.scalar.activation(
                            gth[:, hc, :],
                            pp,
                            Tanh,
                            bias=bgh_sb[:, hc : hc + 1],
                            scale=0.5,
                        )

                    v_sb = vv.tile([P, KC, H, CH], bf16, tag="v")
                    for rc in range(KC):
                        pp = psA.tile([P, SEQ], f32, tag="psA")
                        for cc in range(CC):
                            nc.tensor.matmul(
                                pp[:, 0:HID],
                                xTs["v"][:, cc, P * rc : P * (rc + 1)],
                                w_sbs["wv"][:, cc, :],
                                start=(cc == 0),
                                stop=(cc == CC - 1),
                            )
                        nc.vector.tensor_copy(
                            v_sb[:, rc, :, :],
                            pp[:, 0:HID].rearrange("p (h c) -> p h c", h=H),
                        )

                    # C: attention
                    oT = ot.tile([P, HG, SEQ], bf16, tag="oT")
                    den = dn.tile([H, SEQ], f32, tag="den")
                    for hg in range(HG):
                        Es = {}
                        for pr in range(2):
                            Es[pr] = ee.tile(
                                [P, KC, 2, SEQ], bf16, tag="E", name=f"E_{pr}"
                            )
                            pe_bias = (hg, pr) in PE_PAIRS
                            for kc in range(KC):
                                sp = psQ.tile([P, 2, SEQ], f32, tag="qk", name="qk")
                                for j in range(2):
                                    h2 = 2 * pr + j
                                    nc.tensor.matmul(
                                        sp[:, j, :],
                                        kT[
                                            CH * h2 : CH * (h2 + 1),
                                            hg,
                                            P * kc : P * (kc + 1),
                                        ],
                                        qT[CH * h2 : CH * (h2 + 1), hg, :],
                                        start=True,
                                        stop=not pe_bias,
                                        tile_position=(CH * h2, 0),
                                    )
                                h = 4 * hg + 2 * pr
                                if pe_bias:
                                    # additive bias_pair via identity matmuls
                                    for j in range(2):
                                        nc.tensor.matmul(
                                            sp[:, j, :],
                                            ident_b,
                                            bpt_sb[:, h + j, kc, :],
                                            start=False,
                                            stop=True,
                                        )
                                nc.scalar.activation(
                                    Es[pr][:, kc, :, :],
                                    sp,
                                    Exp,
                                    bias=bm_sb[:, kc, n : n + 1],
                                )
                                if not pe_bias:
                                    eng = (
                                        nc.gpsimd
                                        if (hg, pr, kc) in GPS_TRIPLES
                                        else nc.vector
                                    )
                                    eng.tensor_mul(
                                        Es[pr][:, kc, :, :],
                                        Es[pr][:, kc, :, :],
                                        bpt_sb[:, h : h + 2, kc, :],
                                    )

                        # AV: 4-way column tiling -> packed [4*32, q] output
                        # in one PSUM bank; denominators from a second
                        # column-tiled pass with ones weights.
                        po = psO.tile([P, SEQ], f32, tag="po", name="po")
                        pd = psD.tile([P, SEQ], f32, tag="pd", name="pd")
                        for kc in range(KC):
                            for h2 in range(4):
                                e_rhs = Es[h2 // 2][:, kc, h2 % 2, :]
                                nc.tensor.matmul(
                                    po[CH * h2 : CH * (h2 + 1), :],
                                    v_sb[:, kc, 4 * hg + h2, :],
                                    e_rhs,
                                    start=(kc == 0),
                                    stop=(kc == KC - 1),
                                    tile_position=(0, CH * h2),
                                )
                            for h2 in range(4):
                                e_rhs = Es[h2 // 2][:, kc, h2 % 2, :]
                                nc.tensor.matmul(
                                    pd[CH * h2 : CH * h2 + 1, :],
                                    ones_w[:, h2 : h2 + 1],
                                    e_rhs,
                                    start=(kc == 0),
                                    stop=(kc == KC - 1),
                                    tile_position=(0, CH * h2),
                                )
                        nc.vector.tensor_copy(oT[:, hg, :], po)
                        pds = ot.tile([P, SEQ], f32, tag="pds", name="pds")
                        nc.vector.tensor_copy(pds, pd)
                        nc.sync.dma_start(
                            out=den[4 * hg : 4 * (hg + 1), :],
                            in_=pds[0:P:CH, :],
                        )
                        if "es" in dbg and hg == 0:
                            ef = ot.tile([P, KC, 2, SEQ], f32, tag="ef", name="ef")
                            nc.vector.tensor_copy(ef, Es[0])
                            nc.sync.dma_start(out=dbg["es"][n], in_=ef[:, 0])

                    if "ot" in dbg:
                        otf = ot.tile([P, HG, SEQ], f32, tag="otf", name="otf")
                        nc.vector.tensor_copy(otf, oT)
                        nc.sync.dma_start(out=dbg["ot"][n], in_=otf)
                    if "den" in dbg:
                        nc.sync.dma_start(out=dbg["den"][n], in_=den)

                    return (n, oT, den, gth)

                def emit_tail_head(state):
                    # D1: normalize + gate fold. Emitted right after the
                    # row's front so the serial chain recip -> broadcast ->
                    # fold -> gate overlaps the NEXT row's front entirely.
                    n, oT, den, gth = state
                    rden = dn.tile([H, SEQ], f32, tag="rden")
                    nc.vector.reciprocal(rden, den)
                    dscr = drp.tile([H, SEQ], f32, tag="dscr")
                    nc.sync.dma_start(out=dscr, in_=rden)
                    rbc = gp.tile([P, HG, SEQ], f32, tag="rbc")
                    oTg = gp.tile([P, HG, SEQ], bf16, tag="oTg")
                    for h in range(H):
                        nc.sync.dma_start(
                            out=rbc[CH * (h % 4) : CH * (h % 4 + 1), h // 4, :],
                            in_=dscr[h : h + 1, :].to_broadcast([CH, SEQ]),
                        )
                    for hc in range(HC):
                        nc.gpsimd.tensor_scalar(
                            gth[:, hc, :], gth[:, hc, :], 0.5, 0.5, MULT, ADD
                        )
                        nc.gpsimd.tensor_mul(
                            rbc[:, hc, :], rbc[:, hc, :], gth[:, hc, :]
                        )
                        nc.vector.tensor_mul(
                            oTg[:, hc, :], oT[:, hc, :], rbc[:, hc, :]
                        )
                    if "rbc" in dbg:
                        rbf = ot.tile([P, HG, SEQ], f32, tag="rbf", name="rbf")
                        nc.vector.tensor_copy(rbf, rbc)
                        nc.sync.dma_start(out=dbg["rbc"][n], in_=rbf)
                    if "otg" in dbg:
                        ogf = ot.tile([P, HG, SEQ], f32, tag="ogf", name="ogf")
                        nc.vector.tensor_copy(ogf, oTg)
                        nc.sync.dma_start(out=dbg["otg"][n], in_=ogf)
                    return (n, oTg)

                def emit_tail_tail(state):
                    # D2: output projection, emitted one row late.
                    n, oTg = state
                    for qc in range(QC):
                        pp = psA.tile([P, SEQ], f32, tag="psA")
                        for hc in range(HC):
                            nc.tensor.matmul(
                                pp[:, 0:C],
                                oTg[:, hc, P * qc : P * (qc + 1)],
                                wo_sb[:, hc, :],
                                start=(hc == 0),
                                stop=(hc == HC - 1),
                            )
                        osb = ou.tile([P, C], f32, tag="osb")
                        nc.vector.tensor_add(osb, pp[:, 0:C], bo_sb)
                        if "osb" in dbg and qc == 0:
                            nc.sync.dma_start(out=dbg["osb"][n], in_=osb)
                        nc.sync.dma_start(
                            out=out[n, P * qc : P * (qc + 1), :], in_=osb
                        )

                pending = None
                for n in range(NL):
                    state = emit_front(n, xTs0 if n == 0 else None)
                    head = emit_tail_head(state)
                    if pending is not None:
                        emit_tail_tail(pending)
                    pending = head
                emit_tail_tail(pending)

    return nc


_NC_CACHE = None


def _get_nc():
    global _NC_CACHE
    if _NC_CACHE is None:
        _NC_CACHE = _build_nc()
    return _NC_CACHE


def _to_bf16(a):
    import ml_dtypes

    return np.asarray(a, dtype=ml_dtypes.bfloat16)


def _prepare_in_maps(q_x, k_x, v_x, bias_mask, bias_pair, wq, wk, wv, wg, bg, wo, bo):
    wq_s = np.ascontiguousarray(wq / math.sqrt(CH), dtype=np.float32)
    bpt = np.ascontiguousarray(
        np.transpose(bias_pair[0, 0], (0, 2, 1)), dtype=np.float32
    )  # [h, k, q]
    # heads in PE_PAIRS use the additive PE-identity path: raw bias;
    # the rest are multiplicative: exp(bias)
    _pe_heads = {4 * hg + 2 * pr + j for (hg, pr) in PE_PAIRS for j in range(2)}
    for _h in range(H):
        if _h not in _pe_heads:
            bpt[_h] = np.exp(bpt[_h])
    bpt_exp = _to_bf16(bpt)
    bgh = np.ascontiguousarray((bg / 2.0).reshape(HC, P).T, dtype=np.float32)
    bo_bc = np.ascontiguousarray(np.tile(bo[None, :], (P, 1)), dtype=np.float32)
    bm_all = np.asarray(bias_mask[0, :, 0, 0, :], dtype=np.float32)  # [64, 512]
    wo_b = _to_bf16(wo)

    in_maps = []
    for c in range(N_CORES):
        ns = slice(NL * c, NL * (c + 1))
        bm_r = np.ascontiguousarray(
            bm_all[ns].reshape(NL, KC, P).transpose(2, 1, 0), dtype=np.float32
        )
        in_maps.append(
            {
                "qx": np.ascontiguousarray(
                    q_x[0, ns].transpose(0, 2, 1), dtype=np.float32
                ),
                "kx": np.ascontiguousarray(
                    k_x[0, ns].transpose(0, 2, 1), dtype=np.float32
                ),
                "vx": np.ascontiguousarray(
                    v_x[0, ns].transpose(0, 2, 1), dtype=np.float32
                ),
                "bpt": bpt_exp,
                "bm": bm_r,
                "wq": wq_s,
                "wk": np.ascontiguousarray(wk, dtype=np.float32),
                "wv": np.ascontiguousarray(wv, dtype=np.float32),
                "wg": np.ascontiguousarray(wg, dtype=np.float32),
                "bgh": bgh,
                "wo": wo_b,
                "bo_bc": bo_bc,
            }
        )
    return in_maps


def run(trace=False, **inputs):
    """Run the kernel; returns (output, BassKernelResults)."""
    args = {k: np.asarray(v) for k, v in inputs.items()}
    in_maps = _prepare_in_maps(
        args["q_x"], args["k_x"], args["v_x"], args["bias_mask"],
        args["bias_pair"], args["wq"], args["wk"], args["wv"], args["wg"],
        args["bg"], args["wo"], args["bo"],
    )
    nc = _get_nc()
    res = run_bass_kernel_spmd(nc, in_maps, list(range(N_CORES)), trace=trace)
    out = np.empty((1, NL * N_CORES, SEQ, C), dtype=np.float32)
    for c in range(N_CORES):
        out[0, NL * c : NL * (c + 1)] = res.results[c]["out"]
    return out, res


def kernel(**inputs):
    out, _ = run(trace=False, **inputs)
    return out


if __name__ == "__main__":
    rng = np.random.default_rng(0)
    demo = {
        "q_x": rng.standard_normal((1, 64, SEQ, C)).astype(np.float32),
        "k_x": rng.standard_normal((1, 64, SEQ, C)).astype(np.float32),
        "v_x": rng.standard_normal((1, 64, SEQ, C)).astype(np.float32),
        "bias_mask": rng.standard_normal((1, 64, 1, 1, SEQ)).astype(np.float32),
        "bias_pair": rng.standard_normal((1, 1, H, SEQ, SEQ)).astype(np.float32),
        "wq": (rng.standard_normal((C, HID)) / 16).astype(np.float32),
        "wk": (rng.standard_normal((C, HID)) / 16).astype(np.float32),
        "wv": (rng.standard_normal((C, HID)) / 16).astype(np.float32),
        "wg": (rng.standard_normal((C, HID)) * 0.02).astype(np.float32),
        "bg": np.ones((HID,), dtype=np.float32),
        "wo": (rng.standard_normal((HID, C)) * 0.02).astype(np.float32),
        "bo": np.zeros((C,), dtype=np.float32),
    }
    o = kernel(**demo)
    print("kernel ran, out shape", o.shape, "mean", float(np.abs(o).mean()))


# revision 28
# speedup vs baseline: 1.2511x; 1.2511x over previous
"""Trainium2 Bass kernel for AlphaFold-style gated MSA attention.

Reference computation (per batch b=1, per MSA row n of 64):
    q = (q_x @ wq) / sqrt(32);  k = k_x @ wk;  v = v_x @ wv      (heads: 8 x 32)
    a = softmax(q k^T + bias_mask[n,k] + bias_pair[h,q,k])
    o = (a @ v) * sigmoid(q_x @ wg + bg)
    out = o @ wo + bo

Distribution: data-parallel over the 64 MSA rows -> 8 rows per NeuronCore.

Per-core schedule (per row n):
  1. Projections in f32r from pre-transposed [C, seq] inputs; PSUM results
     are cast to bf16 on evacuation (qT/kT/v for the bf16 attention
     matmuls; the gate tanh comes out of ACT directly in bf16).
  2. S^T = k_h q_h^T per head-pair/key-chunk (bf16, K=32) using 2-way PE
     row tiling: the two matmuls of a pair target disjoint 32-row bands
     (tile_position) and distinct PSUM banks, so they stream concurrently.
     bias_mask folds into the ACT exp as a per-partition bias (S^T layout
     puts keys on partitions); softmax max-subtraction is skipped (logits
     are O(5), far from overflow). exp writes bf16.
  3. bias_pair is applied multiplicatively: the host ships exp(bias_pair)
     in bf16 and E *= expBP runs on DVE (2x bf16 mode) for 6 heads and
     GPSIMD for 2 heads (engine balance).
  4. o^T = V^T E per head with 4-way PE column tiling: head h2 writes PSUM
     partitions 32*h2 of ONE bank, so the group's output is born packed
     [4*32, q] - no banding DMAs. The softmax denominators come from a
     second column-tiled pass with ones weights (M=1 per head).
  5. Tail (software-pipelined one row late): reciprocal of denominators,
     broadcast via DRAM round-trip, gate * recip fold (GPSIMD), output
     gating mul (DVE, bf16 2x), then the output projection in bf16.
"""

import math
import os
import sys

for _p in ("/opt/trn_rl_repo", "/root/.axon_site/_ro/trn_rl_repo"):
    if os.path.isdir(_p) and _p not in sys.path:
        sys.path.append(_p)

import numpy as np

import bass_rust
import concourse.bass as bass
import concourse.mybir as mybir
import concourse.tile as tile
from concourse.bass_utils import run_bass_kernel_spmd
from concourse.masks import make_identity
from concourse.tile import ScopedClock

f32 = mybir.dt.float32
f32r = mybir.dt.float32r
bf16 = mybir.dt.bfloat16

N_CORES = 8
NL = 8        # MSA rows per core (64 / 8)
SEQ = 512     # q and k sequence length
C = 256       # channel dim of q_x/k_x/v_x and the output
HID = 256     # heads * c_hidden
H = 8         # heads
CH = 32       # c_hidden per head
P = 128
CC = C // P   # 2 contraction chunks for projections
HC = HID // P  # 2 hidden chunks
KC = SEQ // P  # 4 key chunks
QC = SEQ // P  # 4 query chunks
HG = 2        # head groups of 4

# Engine assignment for the bias_pair application (balance knobs):
# PE_PAIRS get an ADDITIVE bias via identity matmuls into the S PSUM
# (host ships raw bf16 bias_pair for those heads); the rest get the
# multiplicative exp(bias_pair) on DVE, except GPS_TRIPLES (hg, pr, kc)
# which run on GPSIMD.
PE_PAIRS = {(0, 1), (1, 1)}
GPS_TRIPLES = {(0, 0, 1), (1, 0, 3)}


class _TileContextSplitWaits(tile.TileContext):
    """This container's walrus supports ONE sync-wait per instruction (the
    TRN2 EVENTS struct has a single wait slot and this build refuses to
    expand multi-wait instructions). Tile attaches several waits to one
    instruction; split the extras onto same-engine NOPs emitted just before
    it — the engine queue is in-order, so this is semantically identical."""

    def _add_instruction(self, inst):
        si = inst.sync_info
        if (
            si is not None
            and len(si.on_wait) > 1
            and inst.engine != mybir.EngineType.Unassigned
        ):
            waits = list(si.on_wait)
            for w in waits[:-1]:
                nop = mybir.InstNoOp(
                    name=self.nc.get_next_instruction_name(),
                    sync_info=mybir.SyncInfo(on_wait=[w], on_update=[]),
                    bass_nofuse=True,
                    engine=inst.engine,
                )
                super()._add_instruction(nop)
            inst.sync_info = mybir.SyncInfo(
                on_wait=waits[-1:], on_update=list(si.on_update)
            )
        super()._add_instruction(inst)

    def _drain_and_barrier(self, tick_clock, wait_clock):
        nc = self.nc
        drain_inst = nc.sync.drain()
        wait_clock.add_sem_waits(
            drain_inst.ins, ScopedClock({None: tick_clock.global_clock})
        )
        si = drain_inst.ins.sync_info
        if si is not None and len(si.on_wait) > 1:
            waits = list(si.on_wait)
            updates = list(si.on_update)
            drain_inst.ins.sync_info = bass_rust.SyncInfo(
                on_wait=waits[:1], on_update=[]
            )
            for i, w in enumerate(waits[1:]):
                upd = updates if i == len(waits) - 2 else []
                nop = nc.sync.nop()
                nop.ins.sync_info = bass_rust.SyncInfo(on_wait=[w], on_update=upd)
        nc.all_engine_barrier()
        assert self.sems is not None
        popped = nc._tile_sem_poison_stack.pop()
        assert popped is self._sem_poison
        nc.clear_and_free_semaphores(list(self.sems.allocated().values()))
        nc.all_engine_barrier()


def _build_nc():
    nc = bass.Bass(
        "TRN2", target_bir_lowering=False, debug=False, num_devices=N_CORES
    )
    qx = nc.dram_tensor("qx", [NL, C, SEQ], f32r, kind="ExternalInput").ap()
    kx = nc.dram_tensor("kx", [NL, C, SEQ], f32r, kind="ExternalInput").ap()
    vx = nc.dram_tensor("vx", [NL, C, SEQ], f32r, kind="ExternalInput").ap()
    # exp(bias_pair) transposed [h, k, q], bf16
    bpt = nc.dram_tensor("bpt", [H, SEQ, SEQ], bf16, kind="ExternalInput").ap()
    bm = nc.dram_tensor("bm", [P, KC, NL], f32, kind="ExternalInput").ap()
    wq = nc.dram_tensor("wq", [C, HID], f32r, kind="ExternalInput").ap()
    wk = nc.dram_tensor("wk", [C, HID], f32r, kind="ExternalInput").ap()
    wv = nc.dram_tensor("wv", [C, HID], f32r, kind="ExternalInput").ap()
    wg = nc.dram_tensor("wg", [C, HID], f32r, kind="ExternalInput").ap()
    bgh = nc.dram_tensor("bgh", [P, HC], f32, kind="ExternalInput").ap()
    wo = nc.dram_tensor("wo", [HID, C], bf16, kind="ExternalInput").ap()
    bo_bc = nc.dram_tensor("bo_bc", [P, C], f32, kind="ExternalInput").ap()
    out = nc.dram_tensor("out", [NL, SEQ, C], f32, kind="ExternalOutput").ap()
    dbg = {}
    _flags = set(
        f for f in os.environ.get("BASS_DEBUG_OUT", "").split(",") if f
    )
    if "all" in _flags:
        _flags = {"den", "ot", "rbc", "es", "otg", "osb"}
    _shapes = {
        "den": [NL, H, SEQ],
        "ot": [NL, P, HG, SEQ],
        "rbc": [NL, P, HG, SEQ],
        "es": [NL, P, 2, SEQ],
        "otg": [NL, P, HG, SEQ],
        "osb": [NL, P, C],
    }
    for f in _flags:
        dbg[f] = nc.dram_tensor(
            f + "_dbg", _shapes[f], f32, kind="ExternalOutput"
        ).ap()

    Exp = mybir.ActivationFunctionType.Exp
    Tanh = mybir.ActivationFunctionType.Tanh
    MULT = mybir.AluOpType.mult
    ADD = mybir.AluOpType.add

    with _TileContextSplitWaits(nc) as tc:
        with (
            tc.tile_pool(name="const", bufs=1) as const,
            tc.tile_pool(name="dram", bufs=2, space="DRAM") as drp,
        ):
            # --- constants ---------------------------------------------------
            w_sbs = {}
            for name, w_ap in (("wq", wq), ("wk", wk), ("wv", wv), ("wg", wg)):
                w_sbs[name] = const.tile(
                    [P, CC, HID], f32r, tag=f"w_{name}", name=f"w_{name}"
                )
                nc.sync.dma_start(
                    out=w_sbs[name],
                    in_=w_ap.rearrange("(cc p) h -> p cc h", p=P),
                )
            wo_sb = const.tile([P, HC, C], bf16, tag="w_wo")
            nc.sync.dma_start(
                out=wo_sb, in_=wo.rearrange("(hc p) c -> p hc c", p=P)
            )
            bm_sb = const.tile([P, KC, NL], f32, tag="bm")
            nc.sync.dma_start(out=bm_sb, in_=bm)
            bgh_sb = const.tile([P, HC], f32, tag="bgh")
            nc.sync.dma_start(out=bgh_sb, in_=bgh)
            bo_sb = const.tile([P, C], f32, tag="bo")
            nc.sync.dma_start(out=bo_sb, in_=bo_bc)
            ones_w = const.tile([P, 4], bf16, tag="ones_w")
            nc.vector.memset(ones_w, 1.0)
            ones_bc = const.tile([P, CH], bf16, tag="ones_bc")
            nc.vector.memset(ones_bc, 1.0)
            ident_f = const.tile([P, P], f32, tag="ident_f")
            make_identity(nc, ident_f)
            ident_b = const.tile([P, P], bf16, tag="ident_b")
            nc.vector.tensor_copy(ident_b, ident_f)

            # --- main loop ---------------------------------------------------
            with (
                tc.tile_pool(name="xt", bufs=2) as xt,
                tc.tile_pool(name="pj", bufs=1) as pj,
                tc.tile_pool(name="gh", bufs=2) as gh,
                tc.tile_pool(name="vv", bufs=2) as vv,
                tc.tile_pool(name="ee", bufs=4) as ee,
                tc.tile_pool(name="ot", bufs=2) as ot,
                tc.tile_pool(name="dn", bufs=2) as dn,
                tc.tile_pool(name="gp", bufs=2) as gp,
                tc.tile_pool(name="ou", bufs=2) as ou,
                tc.tile_pool(name="psQ", bufs=2, space="PSUM") as psQ,
                tc.tile_pool(name="psO", bufs=1, space="PSUM") as psO,
                tc.tile_pool(name="psD", bufs=1, space="PSUM") as psD,
                tc.tile_pool(name="psA", bufs=2, space="PSUM") as psA,
            ):
                def emit_xt(n):
                    # A: inputs arrive pre-transposed [C, seq], f32r in DRAM.
                    xTs = {}
                    for name, src_ap in (("q", qx), ("k", kx), ("v", vx)):
                        xT = xt.tile([P, CC, SEQ], f32r, tag=f"xt_{name}")
                        nc.sync.dma_start(
                            out=xT,
                            in_=src_ap[n].rearrange("(cc p) s -> p cc s", p=P),
                        )
                        xTs[name] = xT
                    return xTs

                # Row 0's input DMAs go out BEFORE the big bias_pair const
                # load: the SP queue is in-order, so row 0's projections
                # would otherwise wait for the full 4MB bpt transfer.
                xTs0 = emit_xt(0)
                bpt_sb = const.tile([P, H, KC, SEQ], bf16, tag="bpt")
                for h in range(H):
                    nc.sync.dma_start(
                        out=bpt_sb[:, h],
                        in_=bpt[h].rearrange("(kc p) q -> p kc q", p=P),
                    )

                def emit_front(n, xTs=None):
                    if xTs is None:
                        xTs = emit_xt(n)

                    # B: projections (f32r matmuls, bf16 evacuation)
                    qT = pj.tile([P, HC, SEQ], bf16, tag="qT")
                    kT = pj.tile([P, HC, SEQ], bf16, tag="kT")
                    for dst, wname, src in (
                        (qT, "wq", xTs["q"]),
                        (kT, "wk", xTs["k"]),
                    ):
                        for hc in range(HC):
                            pp = psA.tile([P, SEQ], f32, tag="psA")
                            for cc in range(CC):
                                nc.tensor.matmul(
                                    pp,
                                    w_sbs[wname][:, cc, P * hc : P * (hc + 1)],
                                    src[:, cc, :],
                                    start=(cc == 0),
                                    stop=(cc == CC - 1),
                                )
                            nc.vector.tensor_copy(dst[:, hc, :], pp)

                    gth = gh.tile([P, HC, SEQ], bf16, tag="gth")
                    for hc in range(HC):
                        pp = psA.tile([P, SEQ], f32, tag="psA")
                        for cc in range(CC):
                            nc.tensor.matmul(
                                pp,
                                w_sbs["wg"][:, cc, P * hc : P * (hc + 1)],
                                xTs["q"][:, cc, :],
                                start=(cc == 0),
                                stop=(cc == CC - 1),
                            )
                        # sigmoid(x + bg) = 0.5*tanh((x + bg)/2) + 0.5
                        nc.scalar.activation(
                            gth[:, hc, :],
                            pp,
                            Tanh,
                            bias=bgh_sb[:, hc : hc + 1],
                            scale=0.5,
                        )

                    v_sb = vv.tile([P, KC, H, CH], bf16, tag="v")
                    for rc in range(KC):
                        pp = psA.tile([P, SEQ], f32, tag="psA")
                        for cc in range(CC):
                            nc.tensor.matmul(
                                pp[:, 0:HID],
                                xTs["v"][:, cc, P * rc : P * (rc + 1)],
                                w_sbs["wv"][:, cc, :],
                                start=(cc == 0),
                                stop=(cc == CC - 1),
                            )
                        nc.vector.tensor_copy(
                            v_sb[:, rc, :, :],
                            pp[:, 0:HID].rearrange("p (h c) -> p h c", h=H),
                        )

                    # C: attention
                    oT = ot.tile([P, HG, SEQ], bf16, tag="oT")
                    den = dn.tile([H, SEQ], f32, tag="den")
                    for hg in range(HG):
                        Es = {}
                        for pr in range(2):
                            Es[pr] = ee.tile(
                                [P, KC, 2, SEQ], bf16, tag="E", name=f"E_{pr}"
                            )
                            pe_bias = (hg, pr) in PE_PAIRS
                            for kc in range(KC):
                                sp = psQ.tile([P, 2, SEQ], f32, tag="qk", name="qk")
                                for j in range(2):
                                    h2 = 2 * pr + j
                                    nc.tensor.matmul(
                                        sp[:, j, :],
                                        kT[
                                            CH * h2 : CH * (h2 + 1),
                                            hg,
                                            P * kc : P * (kc + 1),
                                        ],
                                        qT[CH * h2 : CH * (h2 + 1), hg, :],
                                        start=True,
                                        stop=not pe_bias,
                                        tile_position=(CH * h2, 0),
                                    )
                                h = 4 * hg + 2 * pr
                                if pe_bias:
                                    # additive bias_pair via identity matmuls
                                    for j in range(2):
                                        nc.tensor.matmul(
                                            sp[:, j, :],
                                            ident_b,
                                            bpt_sb[:, h + j, kc, :],
                                            start=False,
                                            stop=True,
                                        )
                                nc.scalar.activation(
                                    Es[pr][:, kc, :, :],
                                    sp,
                                    Exp,
                                    bias=bm_sb[:, kc, n : n + 1],
                                )
                                if not pe_bias:
                                    eng = (
                                        nc.gpsimd
                                        if (hg, pr, kc) in GPS_TRIPLES
                                        else nc.vector
                                    )
                                    eng.tensor_mul(
                                        Es[pr][:, kc, :, :],
                                        Es[pr][:, kc, :, :],
                                        bpt_sb[:, h : h + 2, kc, :],
                                    )

                        # AV: 4-way column tiling -> packed [4*32, q] output
                        # in one PSUM bank; denominators from a second
                        # column-tiled pass with ones weights.
                        po = psO.tile([P, SEQ], f32, tag="po", name="po")
                        pd = psD.tile([P, SEQ], f32, tag="pd", name="pd")
                        for kc in range(KC):
                            for h2 in range(4):
                                e_rhs = Es[h2 // 2][:, kc, h2 % 2, :]
                                nc.tensor.matmul(
                                    po[CH * h2 : CH * (h2 + 1), :],
                                    v_sb[:, kc, 4 * hg + h2, :],
                                    e_rhs,
                                    start=(kc == 0),
                                    stop=(kc == KC - 1),
                                    tile_position=(0, CH * h2),
                                )
                            for h2 in range(4):
                                e_rhs = Es[h2 // 2][:, kc, h2 % 2, :]
                                nc.tensor.matmul(
                                    pd[CH * h2 : CH * h2 + 1, :],
                                    ones_w[:, h2 : h2 + 1],
                                    e_rhs,
                                    start=(kc == 0),
                                    stop=(kc == KC - 1),
                                    tile_position=(0, CH * h2),
                                )
                        nc.vector.tensor_copy(oT[:, hg, :], po)
                        pds = ot.tile([P, SEQ], f32, tag="pds", name="pds")
                        nc.vector.tensor_copy(pds, pd)
                        nc.sync.dma_start(
                            out=den[4 * hg : 4 * (hg + 1), :],
                            in_=pds[0:P:CH, :],
                        )
                        if "es" in dbg and hg == 0:
                            ef = ot.tile([P, KC, 2, SEQ], f32, tag="ef", name="ef")
                            nc.vector.tensor_copy(ef, Es[0])
                            nc.sync.dma_start(out=dbg["es"][n], in_=ef[:, 0])

                    if "ot" in dbg:
                        otf = ot.tile([P, HG, SEQ], f32, tag="otf", name="otf")
                        nc.vector.tensor_copy(otf, oT)
                        nc.sync.dma_start(out=dbg["ot"][n], in_=otf)
                    if "den" in dbg:
                        nc.sync.dma_start(out=dbg["den"][n], in_=den)

                    return (n, oT, den, gth)

                def emit_tail_head(state):
                    # D1: normalize + gate fold. Emitted right after the
                    # row's front so the serial chain recip -> broadcast ->
                    # fold -> gate overlaps the NEXT row's front entirely.
                    n, oT, den, gth = state
                    rden = dn.tile([H, SEQ], f32, tag="rden")
                    nc.vector.reciprocal(rden, den)
                    dscr = drp.tile([H, SEQ], f32, tag="dscr")
                    nc.sync.dma_start(out=dscr, in_=rden)
                    rbc = gp.tile([P, HG, SEQ], f32, tag="rbc")
                    oTg = gp.tile([P, HG, SEQ], bf16, tag="oTg")
                    for h in range(H):
                        nc.sync.dma_start(
                            out=rbc[CH * (h % 4) : CH * (h % 4 + 1), h // 4, :],
                            in_=dscr[h : h + 1, :].to_broadcast([CH, SEQ]),
                        )
                    for hc in range(HC):
                        nc.gpsimd.tensor_scalar(
                            gth[:, hc, :], gth[:, hc, :], 0.5, 0.5, MULT, ADD
                        )
                        nc.gpsimd.tensor_mul(
                            rbc[:, hc, :], rbc[:, hc, :], gth[:, hc, :]
                        )
                        nc.vector.tensor_mul(
                            oTg[:, hc, :], oT[:, hc, :], rbc[:, hc, :]
                        )
                    if "rbc" in dbg:
                        rbf = ot.tile([P, HG, SEQ], f32, tag="rbf", name="rbf")
                        nc.vector.tensor_copy(rbf, rbc)
                        nc.sync.dma_start(out=dbg["rbc"][n], in_=rbf)
                    if "otg" in dbg:
                        ogf = ot.tile([P, HG, SEQ], f32, tag="ogf", name="ogf")
                        nc.vector.tensor_copy(ogf, oTg)
                        nc.sync.dma_start(out=dbg["otg"][n], in_=ogf)
                    return (n, oTg)

                def emit_tail_tail(state):
                    # D2: output projection, emitted one row late.
                    n, oTg = state
                    for qc in range(QC):
                        pp = psA.tile([P, SEQ], f32, tag="psA")
                        for hc in range(HC):
                            nc.tensor.matmul(
                                pp[:, 0:C],
                                oTg[:, hc, P * qc : P * (qc + 1)],
                                wo_sb[:, hc, :],
                                start=(hc == 0),
                                stop=(hc == HC - 1),
                            )
                        osb = ou.tile([P, C], f32, tag="osb")
                        nc.vector.tensor_add(osb, pp[:, 0:C], bo_sb)
                        if "osb" in dbg and qc == 0:
                            nc.sync.dma_start(out=dbg["osb"][n], in_=osb)
                        nc.sync.dma_start(
                            out=out[n, P * qc : P * (qc + 1), :], in_=osb
                        )

                pending = None
                for n in range(NL):
                    state = emit_front(n, xTs0 if n == 0 else None)
                    head = emit_tail_head(state)
                    if pending is not None:
                        emit_tail_tail(pending)
                    pending = head
                emit_tail_tail(pending)

    return nc


_NC_CACHE = None


def _get_nc():
    global _NC_CACHE
    if _NC_CACHE is None:
        _NC_CACHE = _build_nc()
    return _NC_CACHE


def _to_bf16(a):
    import ml_dtypes

    return np.asarray(a, dtype=ml_dtypes.bfloat16)


def _prepare_in_maps(q_x, k_x, v_x, bias_mask, bias_pair, wq, wk, wv, wg, bg, wo, bo):
    wq_s = np.ascontiguousarray(wq / math.sqrt(CH), dtype=np.float32)
    bpt = np.ascontiguousarray(
        np.transpose(bias_pair[0, 0], (0, 2, 1)), dtype=np.float32
    )  # [h, k, q]
    # heads in PE_PAIRS use the additive PE-identity path: raw bias;
    # the rest are multiplicative: exp(bias)
    _pe_heads = {4 * hg + 2 * pr + j for (hg, pr) in PE_PAIRS for j in range(2)}
    for _h in range(H):
        if _h not in _pe_heads:
            bpt[_h] = np.exp(bpt[_h])
    bpt_exp = _to_bf16(bpt)
    bgh = np.ascontiguousarray((bg / 2.0).reshape(HC, P).T, dtype=np.float32)
    bo_bc = np.ascontiguousarray(np.tile(bo[None, :], (P, 1)), dtype=np.float32)
    bm_all = np.asarray(bias_mask[0, :, 0, 0, :], dtype=np.float32)  # [64, 512]
    wo_b = _to_bf16(wo)

    in_maps = []
    for c in range(N_CORES):
        ns = slice(NL * c, NL * (c + 1))
        bm_r = np.ascontiguousarray(
            bm_all[ns].reshape(NL, KC, P).transpose(2, 1, 0), dtype=np.float32
        )
        in_maps.append(
            {
                "qx": np.ascontiguousarray(
                    q_x[0, ns].transpose(0, 2, 1), dtype=np.float32
                ),
                "kx": np.ascontiguousarray(
                    k_x[0, ns].transpose(0, 2, 1), dtype=np.float32
                ),
                "vx": np.ascontiguousarray(
                    v_x[0, ns].transpose(0, 2, 1), dtype=np.float32
                ),
                "bpt": bpt_exp,
                "bm": bm_r,
                "wq": wq_s,
                "wk": np.ascontiguousarray(wk, dtype=np.float32),
                "wv": np.ascontiguousarray(wv, dtype=np.float32),
                "wg": np.ascontiguousarray(wg, dtype=np.float32),
                "bgh": bgh,
                "wo": wo_b,
                "bo_bc": bo_bc,
            }
        )
    return in_maps


def run(trace=False, **inputs):
    """Run the kernel; returns (output, BassKernelResults)."""
    args = {k: np.asarray(v) for k, v in inputs.items()}
    in_maps = _prepare_in_maps(
        args["q_x"], args["k_x"], args["v_x"], args["bias_mask"],
        args["bias_pair"], args["wq"], args["wk"], args["wv"], args["wg"],
        args["bg"], args["wo"], args["bo"],
    )
    nc = _get_nc()
    res = run_bass_kernel_spmd(nc, in_maps, list(range(N_CORES)), trace=trace)
    out = np.empty((1, NL * N_CORES, SEQ, C), dtype=np.float32)
    for c in range(N_CORES):
        out[0, NL * c : NL * (c + 1)] = res.results[c]["out"]
    return out, res


def kernel(**inputs):
    out, _ = run(trace=False, **inputs)
    return out


if __name__ == "__main__":
    rng = np.random.default_rng(0)
    demo = {
        "q_x": rng.standard_normal((1, 64, SEQ, C)).astype(np.float32),
        "k_x": rng.standard_normal((1, 64, SEQ, C)).astype(np.float32),
        "v_x": rng.standard_normal((1, 64, SEQ, C)).astype(np.float32),
        "bias_mask": rng.standard_normal((1, 64, 1, 1, SEQ)).astype(np.float32),
        "bias_pair": rng.standard_normal((1, 1, H, SEQ, SEQ)).astype(np.float32),
        "wq": (rng.standard_normal((C, HID)) / 16).astype(np.float32),
        "wk": (rng.standard_normal((C, HID)) / 16).astype(np.float32),
        "wv": (rng.standard_normal((C, HID)) / 16).astype(np.float32),
        "wg": (rng.standard_normal((C, HID)) * 0.02).astype(np.float32),
        "bg": np.ones((HID,), dtype=np.float32),
        "wo": (rng.standard_normal((HID, C)) * 0.02).astype(np.float32),
        "bo": np.zeros((C,), dtype=np.float32),
    }
    o = kernel(**demo)
    print("kernel ran, out shape", o.shape, "mean", float(np.abs(o).mean()))


# revision 29
# speedup vs baseline: 1.2655x; 1.0115x over previous
"""Trainium2 Bass kernel for AlphaFold-style gated MSA attention.

Reference computation (per batch b=1, per MSA row n of 64):
    q = (q_x @ wq) / sqrt(32);  k = k_x @ wk;  v = v_x @ wv      (heads: 8 x 32)
    a = softmax(q k^T + bias_mask[n,k] + bias_pair[h,q,k])
    o = (a @ v) * sigmoid(q_x @ wg + bg)
    out = o @ wo + bo

Distribution: data-parallel over the 64 MSA rows -> 8 rows per NeuronCore.

Per-core schedule (per row n):
  1. Projections in f32r from pre-transposed [C, seq] inputs; PSUM results
     are cast to bf16 on evacuation (qT/kT/v for the bf16 attention
     matmuls; the gate tanh comes out of ACT directly in bf16).
  2. S^T = k_h q_h^T per head-pair/key-chunk (bf16, K=32) using 2-way PE
     row tiling: the two matmuls of a pair target disjoint 32-row bands
     (tile_position) and distinct PSUM banks, so they stream concurrently.
     bias_mask folds into the ACT exp as a per-partition bias (S^T layout
     puts keys on partitions); softmax max-subtraction is skipped (logits
     are O(5), far from overflow). exp writes bf16.
  3. bias_pair is applied multiplicatively: the host ships exp(bias_pair)
     in bf16 and E *= expBP runs on DVE (2x bf16 mode) for 6 heads and
     GPSIMD for 2 heads (engine balance).
  4. o^T = V^T E per head with 4-way PE column tiling: head h2 writes PSUM
     partitions 32*h2 of ONE bank, so the group's output is born packed
     [4*32, q] - no banding DMAs. The softmax denominators come from a
     second column-tiled pass with ones weights (M=1 per head).
  5. Tail (software-pipelined one row late): reciprocal of denominators,
     broadcast via DRAM round-trip, gate * recip fold (GPSIMD), output
     gating mul (DVE, bf16 2x), then the output projection in bf16.
"""

import math
import os
import sys

for _p in ("/opt/trn_rl_repo", "/root/.axon_site/_ro/trn_rl_repo"):
    if os.path.isdir(_p) and _p not in sys.path:
        sys.path.append(_p)

import numpy as np

import bass_rust
import concourse.bass as bass
import concourse.mybir as mybir
import concourse.tile as tile
from concourse.bass_utils import run_bass_kernel_spmd
from concourse.masks import make_identity
from concourse.tile import ScopedClock

f32 = mybir.dt.float32
f32r = mybir.dt.float32r
bf16 = mybir.dt.bfloat16

N_CORES = 8
NL = 8        # MSA rows per core (64 / 8)
SEQ = 512     # q and k sequence length
C = 256       # channel dim of q_x/k_x/v_x and the output
HID = 256     # heads * c_hidden
H = 8         # heads
CH = 32       # c_hidden per head
P = 128
CC = C // P   # 2 contraction chunks for projections
HC = HID // P  # 2 hidden chunks
KC = SEQ // P  # 4 key chunks
QC = SEQ // P  # 4 query chunks
HG = 2        # head groups of 4

# Engine assignment for the bias_pair application (balance knobs):
# PE_PAIRS get an ADDITIVE bias via identity matmuls into the S PSUM
# (host ships raw bf16 bias_pair for those heads); the rest get the
# multiplicative exp(bias_pair) on DVE, except GPS_TRIPLES (hg, pr, kc)
# which run on GPSIMD.
PE_PAIRS = set()
GPS_TRIPLES = set()


class _TileContextSplitWaits(tile.TileContext):
    """This container's walrus supports ONE sync-wait per instruction (the
    TRN2 EVENTS struct has a single wait slot and this build refuses to
    expand multi-wait instructions). Tile attaches several waits to one
    instruction; split the extras onto same-engine NOPs emitted just before
    it — the engine queue is in-order, so this is semantically identical."""

    def _add_instruction(self, inst):
        si = inst.sync_info
        if (
            si is not None
            and len(si.on_wait) > 1
            and inst.engine != mybir.EngineType.Unassigned
        ):
            waits = list(si.on_wait)
            for w in waits[:-1]:
                nop = mybir.InstNoOp(
                    name=self.nc.get_next_instruction_name(),
                    sync_info=mybir.SyncInfo(on_wait=[w], on_update=[]),
                    bass_nofuse=True,
                    engine=inst.engine,
                )
                super()._add_instruction(nop)
            inst.sync_info = mybir.SyncInfo(
                on_wait=waits[-1:], on_update=list(si.on_update)
            )
        super()._add_instruction(inst)

    def _drain_and_barrier(self, tick_clock, wait_clock):
        nc = self.nc
        drain_inst = nc.sync.drain()
        wait_clock.add_sem_waits(
            drain_inst.ins, ScopedClock({None: tick_clock.global_clock})
        )
        si = drain_inst.ins.sync_info
        if si is not None and len(si.on_wait) > 1:
            waits = list(si.on_wait)
            updates = list(si.on_update)
            drain_inst.ins.sync_info = bass_rust.SyncInfo(
                on_wait=waits[:1], on_update=[]
            )
            for i, w in enumerate(waits[1:]):
                upd = updates if i == len(waits) - 2 else []
                nop = nc.sync.nop()
                nop.ins.sync_info = bass_rust.SyncInfo(on_wait=[w], on_update=upd)
        nc.all_engine_barrier()
        assert self.sems is not None
        popped = nc._tile_sem_poison_stack.pop()
        assert popped is self._sem_poison
        nc.clear_and_free_semaphores(list(self.sems.allocated().values()))
        nc.all_engine_barrier()


def _build_nc():
    nc = bass.Bass(
        "TRN2", target_bir_lowering=False, debug=False, num_devices=N_CORES
    )
    qx = nc.dram_tensor("qx", [NL, C, SEQ], f32r, kind="ExternalInput").ap()
    kx = nc.dram_tensor("kx", [NL, C, SEQ], f32r, kind="ExternalInput").ap()
    vx = nc.dram_tensor("vx", [NL, C, SEQ], f32r, kind="ExternalInput").ap()
    # exp(bias_pair) transposed [h, k, q], bf16
    bpt = nc.dram_tensor("bpt", [H, SEQ, SEQ], bf16, kind="ExternalInput").ap()
    bm = nc.dram_tensor("bm", [P, KC, NL], f32, kind="ExternalInput").ap()
    wq = nc.dram_tensor("wq", [C, HID], f32r, kind="ExternalInput").ap()
    wk = nc.dram_tensor("wk", [C, HID], f32r, kind="ExternalInput").ap()
    wv = nc.dram_tensor("wv", [C, HID], f32r, kind="ExternalInput").ap()
    wg = nc.dram_tensor("wg", [C, HID], f32r, kind="ExternalInput").ap()
    bgh = nc.dram_tensor("bgh", [P, HC], f32, kind="ExternalInput").ap()
    wo = nc.dram_tensor("wo", [HID, C], bf16, kind="ExternalInput").ap()
    bo_bc = nc.dram_tensor("bo_bc", [P, C], f32, kind="ExternalInput").ap()
    out = nc.dram_tensor("out", [NL, SEQ, C], f32, kind="ExternalOutput").ap()
    dbg = {}
    _flags = set(
        f for f in os.environ.get("BASS_DEBUG_OUT", "").split(",") if f
    )
    if "all" in _flags:
        _flags = {"den", "ot", "rbc", "es", "otg", "osb"}
    _shapes = {
        "den": [NL, H, SEQ],
        "ot": [NL, P, HG, SEQ],
        "rbc": [NL, P, HG, SEQ],
        "es": [NL, P, 2, SEQ],
        "otg": [NL, P, HG, SEQ],
        "osb": [NL, P, C],
    }
    for f in _flags:
        dbg[f] = nc.dram_tensor(
            f + "_dbg", _shapes[f], f32, kind="ExternalOutput"
        ).ap()

    Exp = mybir.ActivationFunctionType.Exp
    Tanh = mybir.ActivationFunctionType.Tanh
    MULT = mybir.AluOpType.mult
    ADD = mybir.AluOpType.add

    with _TileContextSplitWaits(nc) as tc:
        with (
            tc.tile_pool(name="const", bufs=1) as const,
            tc.tile_pool(name="dram", bufs=2, space="DRAM") as drp,
        ):
            # --- constants ---------------------------------------------------
            w_sbs = {}
            for name, w_ap in (("wq", wq), ("wk", wk), ("wv", wv), ("wg", wg)):
                w_sbs[name] = const.tile(
                    [P, CC, HID], f32r, tag=f"w_{name}", name=f"w_{name}"
                )
                nc.sync.dma_start(
                    out=w_sbs[name],
                    in_=w_ap.rearrange("(cc p) h -> p cc h", p=P),
                )
            wo_sb = const.tile([P, HC, C], bf16, tag="w_wo")
            nc.sync.dma_start(
                out=wo_sb, in_=wo.rearrange("(hc p) c -> p hc c", p=P)
            )
            bm_sb = const.tile([P, KC, NL], f32, tag="bm")
            nc.sync.dma_start(out=bm_sb, in_=bm)
            bgh_sb = const.tile([P, HC], f32, tag="bgh")
            nc.sync.dma_start(out=bgh_sb, in_=bgh)
            bo_sb = const.tile([P, C], f32, tag="bo")
            nc.sync.dma_start(out=bo_sb, in_=bo_bc)
            ones_w = const.tile([P, 4], bf16, tag="ones_w")
            nc.vector.memset(ones_w, 1.0)
            ones_bc = const.tile([P, CH], bf16, tag="ones_bc")
            nc.vector.memset(ones_bc, 1.0)
            ident_f = const.tile([P, P], f32, tag="ident_f")
            make_identity(nc, ident_f)
            ident_b = const.tile([P, P], bf16, tag="ident_b")
            nc.vector.tensor_copy(ident_b, ident_f)

            # --- main loop ---------------------------------------------------
            with (
                tc.tile_pool(name="xt", bufs=2) as xt,
                tc.tile_pool(name="pj", bufs=1) as pj,
                tc.tile_pool(name="gh", bufs=2) as gh,
                tc.tile_pool(name="vv", bufs=2) as vv,
                tc.tile_pool(name="ee", bufs=4) as ee,
                tc.tile_pool(name="ot", bufs=2) as ot,
                tc.tile_pool(name="dn", bufs=2) as dn,
                tc.tile_pool(name="gp", bufs=2) as gp,
                tc.tile_pool(name="ou", bufs=2) as ou,
                tc.tile_pool(name="psQ", bufs=2, space="PSUM") as psQ,
                tc.tile_pool(name="psO", bufs=1, space="PSUM") as psO,
                tc.tile_pool(name="psD", bufs=1, space="PSUM") as psD,
                tc.tile_pool(name="psA", bufs=2, space="PSUM") as psA,
            ):
                def emit_xt(n):
                    # A: inputs arrive pre-transposed [C, seq], f32r in DRAM.
                    xTs = {}
                    for name, src_ap in (("q", qx), ("k", kx), ("v", vx)):
                        xT = xt.tile([P, CC, SEQ], f32r, tag=f"xt_{name}")
                        nc.sync.dma_start(
                            out=xT,
                            in_=src_ap[n].rearrange("(cc p) s -> p cc s", p=P),
                        )
                        xTs[name] = xT
                    return xTs

                # Row 0's input DMAs go out BEFORE the big bias_pair const
                # load: the SP queue is in-order, so row 0's projections
                # would otherwise wait for the full 4MB bpt transfer.
                xTs0 = emit_xt(0)
                bpt_sb = const.tile([P, H, KC, SEQ], bf16, tag="bpt")
                for h in range(H):
                    nc.sync.dma_start(
                        out=bpt_sb[:, h],
                        in_=bpt[h].rearrange("(kc p) q -> p kc q", p=P),
                    )

                def emit_front(n, xTs=None):
                    if xTs is None:
                        xTs = emit_xt(n)

                    # B: projections (f32r matmuls, bf16 evacuation)
                    qT = pj.tile([P, HC, SEQ], bf16, tag="qT")
                    kT = pj.tile([P, HC, SEQ], bf16, tag="kT")
                    for dst, wname, src in (
                        (qT, "wq", xTs["q"]),
                        (kT, "wk", xTs["k"]),
                    ):
                        for hc in range(HC):
                            pp = psA.tile([P, SEQ], f32, tag="psA")
                            for cc in range(CC):
                                nc.tensor.matmul(
                                    pp,
                                    w_sbs[wname][:, cc, P * hc : P * (hc + 1)],
                                    src[:, cc, :],
                                    start=(cc == 0),
                                    stop=(cc == CC - 1),
                                )
                            nc.vector.tensor_copy(dst[:, hc, :], pp)

                    gth = gh.tile([P, HC, SEQ], bf16, tag="gth")
                    for hc in range(HC):
                        pp = psA.tile([P, SEQ], f32, tag="psA")
                        for cc in range(CC):
                            nc.tensor.matmul(
                                pp,
                                w_sbs["wg"][:, cc, P * hc : P * (hc + 1)],
                                xTs["q"][:, cc, :],
                                start=(cc == 0),
                                stop=(cc == CC - 1),
                            )
                        # sigmoid(x + bg) = 0.5*tanh((x + bg)/2) + 0.5
                        nc.scalar.activation(
                            gth[:, hc, :],
                            pp,
                            Tanh,
                            bias=bgh_sb[:, hc : hc + 1],
                            scale=0.5,
                        )

                    v_sb = vv.tile([P, KC, H, CH], bf16, tag="v")
                    for rc in range(KC):
                        pp = psA.tile([P, SEQ], f32, tag="psA")
                        for cc in range(CC):
                            nc.tensor.matmul(
                                pp[:, 0:HID],
                                xTs["v"][:, cc, P * rc : P * (rc + 1)],
                                w_sbs["wv"][:, cc, :],
                                start=(cc == 0),
                                stop=(cc == CC - 1),
                            )
                        nc.vector.tensor_copy(
                            v_sb[:, rc, :, :],
                            pp[:, 0:HID].rearrange("p (h c) -> p h c", h=H),
                        )

                    # C: attention
                    oT = ot.tile([P, HG, SEQ], bf16, tag="oT")
                    den = dn.tile([H, SEQ], f32, tag="den")
                    for hg in range(HG):
                        Es = {}
                        for pr in range(2):
                            Es[pr] = ee.tile(
                                [P, KC, 2, SEQ], bf16, tag="E", name=f"E_{pr}"
                            )
                            pe_bias = (hg, pr) in PE_PAIRS
                            for kc in range(KC):
                                sp = psQ.tile([P, 2, SEQ], f32, tag="qk", name="qk")
                                for j in range(2):
                                    h2 = 2 * pr + j
                                    nc.tensor.matmul(
                                        sp[:, j, :],
                                        kT[
                                            CH * h2 : CH * (h2 + 1),
                                            hg,
                                            P * kc : P * (kc + 1),
                                        ],
                                        qT[CH * h2 : CH * (h2 + 1), hg, :],
                                        start=True,
                                        stop=not pe_bias,
                                        tile_position=(CH * h2, 0),
                                    )
                                h = 4 * hg + 2 * pr
                                if pe_bias:
                                    # additive bias_pair via identity matmuls
                                    for j in range(2):
                                        nc.tensor.matmul(
                                            sp[:, j, :],
                                            ident_b,
                                            bpt_sb[:, h + j, kc, :],
                                            start=False,
                                            stop=True,
                                        )
                                nc.scalar.activation(
                                    Es[pr][:, kc, :, :],
                                    sp,
                                    Exp,
                                    bias=bm_sb[:, kc, n : n + 1],
                                )
                                if not pe_bias:
                                    eng = (
                                        nc.gpsimd
                                        if (hg, pr, kc) in GPS_TRIPLES
                                        else nc.vector
                                    )
                                    eng.tensor_mul(
                                        Es[pr][:, kc, :, :],
                                        Es[pr][:, kc, :, :],
                                        bpt_sb[:, h : h + 2, kc, :],
                                    )

                        # AV: 4-way column tiling -> packed [4*32, q] output
                        # in one PSUM bank; denominators from a second
                        # column-tiled pass with ones weights.
                        po = psO.tile([P, SEQ], f32, tag="po", name="po")
                        pd = psD.tile([P, SEQ], f32, tag="pd", name="pd")
                        for kc in range(KC):
                            for h2 in range(4):
                                e_rhs = Es[h2 // 2][:, kc, h2 % 2, :]
                                nc.tensor.matmul(
                                    po[CH * h2 : CH * (h2 + 1), :],
                                    v_sb[:, kc, 4 * hg + h2, :],
                                    e_rhs,
                                    start=(kc == 0),
                                    stop=(kc == KC - 1),
                                    tile_position=(0, CH * h2),
                                )
                            for h2 in range(4):
                                e_rhs = Es[h2 // 2][:, kc, h2 % 2, :]
                                nc.tensor.matmul(
                                    pd[CH * h2 : CH * h2 + 1, :],
                                    ones_w[:, h2 : h2 + 1],
                                    e_rhs,
                                    start=(kc == 0),
                                    stop=(kc == KC - 1),
                                    tile_position=(0, CH * h2),
                                )
                        nc.vector.tensor_copy(oT[:, hg, :], po)
                        pds = ot.tile([P, SEQ], f32, tag="pds", name="pds")
                        nc.vector.tensor_copy(pds, pd)
                        nc.sync.dma_start(
                            out=den[4 * hg : 4 * (hg + 1), :],
                            in_=pds[0:P:CH, :],
                        )
                        if "es" in dbg and hg == 0:
                            ef = ot.tile([P, KC, 2, SEQ], f32, tag="ef", name="ef")
                            nc.vector.tensor_copy(ef, Es[0])
                            nc.sync.dma_start(out=dbg["es"][n], in_=ef[:, 0])

                    if "ot" in dbg:
                        otf = ot.tile([P, HG, SEQ], f32, tag="otf", name="otf")
                        nc.vector.tensor_copy(otf, oT)
                        nc.sync.dma_start(out=dbg["ot"][n], in_=otf)
                    if "den" in dbg:
                        nc.sync.dma_start(out=dbg["den"][n], in_=den)

                    return (n, oT, den, gth)

                def emit_tail_head(state):
                    # D1: normalize + gate fold. Emitted right after the
                    # row's front so the serial chain recip -> broadcast ->
                    # fold -> gate overlaps the NEXT row's front entirely.
                    n, oT, den, gth = state
                    rden = dn.tile([H, SEQ], f32, tag="rden")
                    nc.vector.reciprocal(rden, den)
                    dscr = drp.tile([H, SEQ], f32, tag="dscr")
                    nc.sync.dma_start(out=dscr, in_=rden)
                    rbc = gp.tile([P, HG, SEQ], f32, tag="rbc")
                    oTg = gp.tile([P, HG, SEQ], bf16, tag="oTg")
                    for h in range(H):
                        nc.sync.dma_start(
                            out=rbc[CH * (h % 4) : CH * (h % 4 + 1), h // 4, :],
                            in_=dscr[h : h + 1, :].to_broadcast([CH, SEQ]),
                        )
                    for hc in range(HC):
                        nc.gpsimd.tensor_scalar(
                            gth[:, hc, :], gth[:, hc, :], 0.5, 0.5, MULT, ADD
                        )
                        nc.gpsimd.tensor_mul(
                            rbc[:, hc, :], rbc[:, hc, :], gth[:, hc, :]
                        )
                        nc.vector.tensor_mul(
                            oTg[:, hc, :], oT[:, hc, :], rbc[:, hc, :]
                        )
                    if "rbc" in dbg:
                        rbf = ot.tile([P, HG, SEQ], f32, tag="rbf", name="rbf")
                        nc.vector.tensor_copy(rbf, rbc)
                        nc.sync.dma_start(out=dbg["rbc"][n], in_=rbf)
                    if "otg" in dbg:
                        ogf = ot.tile([P, HG, SEQ], f32, tag="ogf", name="ogf")
                        nc.vector.tensor_copy(ogf, oTg)
                        nc.sync.dma_start(out=dbg["otg"][n], in_=ogf)
                    return (n, oTg)

                def emit_tail_tail(state):
                    # D2: output projection, emitted one row late.
                    n, oTg = state
                    for qc in range(QC):
                        pp = psA.tile([P, SEQ], f32, tag="psA")
                        for hc in range(HC):
                            nc.tensor.matmul(
                                pp[:, 0:C],
                                oTg[:, hc, P * qc : P * (qc + 1)],
                                wo_sb[:, hc, :],
                                start=(hc == 0),
                                stop=(hc == HC - 1),
                            )
                        osb = ou.tile([P, C], f32, tag="osb")
                        nc.vector.tensor_add(osb, pp[:, 0:C], bo_sb)
                        if "osb" in dbg and qc == 0:
                            nc.sync.dma_start(out=dbg["osb"][n], in_=osb)
                        nc.sync.dma_start(
                            out=out[n, P * qc : P * (qc + 1), :], in_=osb
                        )

                pending = None
                for n in range(NL):
                    state = emit_front(n, xTs0 if n == 0 else None)
                    head = emit_tail_head(state)
                    if pending is not None:
                        emit_tail_tail(pending)
                    pending = head
                emit_tail_tail(pending)

    return nc


_NC_CACHE = None


def _get_nc():
    global _NC_CACHE
    if _NC_CACHE is None:
        _NC_CACHE = _build_nc()
    return _NC_CACHE


def _to_bf16(a):
    import ml_dtypes

    return np.asarray(a, dtype=ml_dtypes.bfloat16)


def _prepare_in_maps(q_x, k_x, v_x, bias_mask, bias_pair, wq, wk, wv, wg, bg, wo, bo):
    wq_s = np.ascontiguousarray(wq / math.sqrt(CH), dtype=np.float32)
    bpt = np.ascontiguousarray(
        np.transpose(bias_pair[0, 0], (0, 2, 1)), dtype=np.float32
    )  # [h, k, q]
    # heads in PE_PAIRS use the additive PE-identity path: raw bias;
    # the rest are multiplicative: exp(bias)
    _pe_heads = {4 * hg + 2 * pr + j for (hg, pr) in PE_PAIRS for j in range(2)}
    for _h in range(H):
        if _h not in _pe_heads:
            bpt[_h] = np.exp(bpt[_h])
    bpt_exp = _to_bf16(bpt)
    bgh = np.ascontiguousarray((bg / 2.0).reshape(HC, P).T, dtype=np.float32)
    bo_bc = np.ascontiguousarray(np.tile(bo[None, :], (P, 1)), dtype=np.float32)
    bm_all = np.asarray(bias_mask[0, :, 0, 0, :], dtype=np.float32)  # [64, 512]
    wo_b = _to_bf16(wo)

    in_maps = []
    for c in range(N_CORES):
        ns = slice(NL * c, NL * (c + 1))
        bm_r = np.ascontiguousarray(
            bm_all[ns].reshape(NL, KC, P).transpose(2, 1, 0), dtype=np.float32
        )
        in_maps.append(
            {
                "qx": np.ascontiguousarray(
                    q_x[0, ns].transpose(0, 2, 1), dtype=np.float32
                ),
                "kx": np.ascontiguousarray(
                    k_x[0, ns].transpose(0, 2, 1), dtype=np.float32
                ),
                "vx": np.ascontiguousarray(
                    v_x[0, ns].transpose(0, 2, 1), dtype=np.float32
                ),
                "bpt": bpt_exp,
                "bm": bm_r,
                "wq": wq_s,
                "wk": np.ascontiguousarray(wk, dtype=np.float32),
                "wv": np.ascontiguousarray(wv, dtype=np.float32),
                "wg": np.ascontiguousarray(wg, dtype=np.float32),
                "bgh": bgh,
                "wo": wo_b,
                "bo_bc": bo_bc,
            }
        )
    return in_maps


def run(trace=False, **inputs):
    """Run the kernel; returns (output, BassKernelResults)."""
    args = {k: np.asarray(v) for k, v in inputs.items()}
    in_maps = _prepare_in_maps(
        args["q_x"], args["k_x"], args["v_x"], args["bias_mask"],
        args["bias_pair"], args["wq"], args["wk"], args["wv"], args["wg"],
        args["bg"], args["wo"], args["bo"],
    )
    nc = _get_nc()
    res = run_bass_kernel_spmd(nc, in_maps, list(range(N_CORES)), trace=trace)
    out = np.empty((1, NL * N_CORES, SEQ, C), dtype=np.float32)
    for c in range(N_CORES):
        out[0, NL * c : NL * (c + 1)] = res.results[c]["out"]
    return out, res


def kernel(**inputs):
    out, _ = run(trace=False, **inputs)
    return out


if __name__ == "__main__":
    rng = np.random.default_rng(0)
    demo = {
        "q_x": rng.standard_normal((1, 64, SEQ, C)).astype(np.float32),
        "k_x": rng.standard_normal((1, 64, SEQ, C)).astype(np.float32),
        "v_x": rng.standard_normal((1, 64, SEQ, C)).astype(np.float32),
        "bias_mask": rng.standard_normal((1, 64, 1, 1, SEQ)).astype(np.float32),
        "bias_pair": rng.standard_normal((1, 1, H, SEQ, SEQ)).astype(np.float32),
        "wq": (rng.standard_normal((C, HID)) / 16).astype(np.float32),
        "wk": (rng.standard_normal((C, HID)) / 16).astype(np.float32),
        "wv": (rng.standard_normal((C, HID)) / 16).astype(np.float32),
        "wg": (rng.standard_normal((C, HID)) * 0.02).astype(np.float32),
        "bg": np.ones((HID,), dtype=np.float32),
        "wo": (rng.standard_normal((HID, C)) * 0.02).astype(np.float32),
        "bo": np.zeros((C,), dtype=np.float32),
    }
    o = kernel(**demo)
    print("kernel ran, out shape", o.shape, "mean", float(np.abs(o).mean()))


# revision 30
# speedup vs baseline: 1.2676x; 1.0017x over previous
"""Trainium2 Bass kernel for AlphaFold-style gated MSA attention.

Reference computation (per batch b=1, per MSA row n of 64):
    q = (q_x @ wq) / sqrt(32);  k = k_x @ wk;  v = v_x @ wv      (heads: 8 x 32)
    a = softmax(q k^T + bias_mask[n,k] + bias_pair[h,q,k])
    o = (a @ v) * sigmoid(q_x @ wg + bg)
    out = o @ wo + bo

Distribution: data-parallel over the 64 MSA rows -> 8 rows per NeuronCore.

Per-core schedule (per row n):
  1. Projections in f32r from pre-transposed [C, seq] inputs; PSUM results
     are cast to bf16 on evacuation (qT/kT/v for the bf16 attention
     matmuls; the gate tanh comes out of ACT directly in bf16).
  2. S^T = k_h q_h^T per head-pair/key-chunk (bf16, K=32) using 2-way PE
     row tiling: the two matmuls of a pair target disjoint 32-row bands
     (tile_position) and distinct PSUM banks, so they stream concurrently.
     bias_mask folds into the ACT exp as a per-partition bias (S^T layout
     puts keys on partitions); softmax max-subtraction is skipped (logits
     are O(5), far from overflow). exp writes bf16.
  3. bias_pair is applied multiplicatively: the host ships exp(bias_pair)
     in bf16 and E *= expBP runs on DVE (2x bf16 mode) for 6 heads and
     GPSIMD for 2 heads (engine balance).
  4. o^T = V^T E per head with 4-way PE column tiling: head h2 writes PSUM
     partitions 32*h2 of ONE bank, so the group's output is born packed
     [4*32, q] - no banding DMAs. The softmax denominators come from a
     second column-tiled pass with ones weights (M=1 per head).
  5. Tail (software-pipelined one row late): reciprocal of denominators,
     broadcast via DRAM round-trip, gate * recip fold (GPSIMD), output
     gating mul (DVE, bf16 2x), then the output projection in bf16.
"""

import math
import os
import sys

for _p in ("/opt/trn_rl_repo", "/root/.axon_site/_ro/trn_rl_repo"):
    if os.path.isdir(_p) and _p not in sys.path:
        sys.path.append(_p)

import numpy as np

import bass_rust
import concourse.bass as bass
import concourse.mybir as mybir
import concourse.tile as tile
from concourse.bass_utils import run_bass_kernel_spmd
from concourse.masks import make_identity
from concourse.tile import ScopedClock

f32 = mybir.dt.float32
f32r = mybir.dt.float32r
bf16 = mybir.dt.bfloat16

N_CORES = 8
NL = 8        # MSA rows per core (64 / 8)
SEQ = 512     # q and k sequence length
C = 256       # channel dim of q_x/k_x/v_x and the output
HID = 256     # heads * c_hidden
H = 8         # heads
CH = 32       # c_hidden per head
P = 128
CC = C // P   # 2 contraction chunks for projections
HC = HID // P  # 2 hidden chunks
KC = SEQ // P  # 4 key chunks
QC = SEQ // P  # 4 query chunks
HG = 2        # head groups of 4

# Engine assignment for the bias_pair application (balance knobs):
# PE_PAIRS get an ADDITIVE bias via identity matmuls into the S PSUM
# (host ships raw bf16 bias_pair for those heads); the rest get the
# multiplicative exp(bias_pair) on DVE, except GPS_TRIPLES (hg, pr, kc)
# which run on GPSIMD.
PE_PAIRS = {(0, 1), (1, 1)}
GPS_TRIPLES = set()


class _TileContextSplitWaits(tile.TileContext):
    """This container's walrus supports ONE sync-wait per instruction (the
    TRN2 EVENTS struct has a single wait slot and this build refuses to
    expand multi-wait instructions). Tile attaches several waits to one
    instruction; split the extras onto same-engine NOPs emitted just before
    it — the engine queue is in-order, so this is semantically identical."""

    def _add_instruction(self, inst):
        si = inst.sync_info
        if (
            si is not None
            and len(si.on_wait) > 1
            and inst.engine != mybir.EngineType.Unassigned
        ):
            waits = list(si.on_wait)
            for w in waits[:-1]:
                nop = mybir.InstNoOp(
                    name=self.nc.get_next_instruction_name(),
                    sync_info=mybir.SyncInfo(on_wait=[w], on_update=[]),
                    bass_nofuse=True,
                    engine=inst.engine,
                )
                super()._add_instruction(nop)
            inst.sync_info = mybir.SyncInfo(
                on_wait=waits[-1:], on_update=list(si.on_update)
            )
        super()._add_instruction(inst)

    def _drain_and_barrier(self, tick_clock, wait_clock):
        nc = self.nc
        drain_inst = nc.sync.drain()
        wait_clock.add_sem_waits(
            drain_inst.ins, ScopedClock({None: tick_clock.global_clock})
        )
        si = drain_inst.ins.sync_info
        if si is not None and len(si.on_wait) > 1:
            waits = list(si.on_wait)
            updates = list(si.on_update)
            drain_inst.ins.sync_info = bass_rust.SyncInfo(
                on_wait=waits[:1], on_update=[]
            )
            for i, w in enumerate(waits[1:]):
                upd = updates if i == len(waits) - 2 else []
                nop = nc.sync.nop()
                nop.ins.sync_info = bass_rust.SyncInfo(on_wait=[w], on_update=upd)
        nc.all_engine_barrier()
        assert self.sems is not None
        popped = nc._tile_sem_poison_stack.pop()
        assert popped is self._sem_poison
        nc.clear_and_free_semaphores(list(self.sems.allocated().values()))
        nc.all_engine_barrier()


def _build_nc():
    nc = bass.Bass(
        "TRN2", target_bir_lowering=False, debug=False, num_devices=N_CORES
    )
    qx = nc.dram_tensor("qx", [NL, C, SEQ], f32r, kind="ExternalInput").ap()
    kx = nc.dram_tensor("kx", [NL, C, SEQ], f32r, kind="ExternalInput").ap()
    vx = nc.dram_tensor("vx", [NL, C, SEQ], f32r, kind="ExternalInput").ap()
    # exp(bias_pair) transposed [h, k, q], bf16
    bpt = nc.dram_tensor("bpt", [H, SEQ, SEQ], bf16, kind="ExternalInput").ap()
    bm = nc.dram_tensor("bm", [P, KC, NL], f32, kind="ExternalInput").ap()
    wq = nc.dram_tensor("wq", [C, HID], f32r, kind="ExternalInput").ap()
    wk = nc.dram_tensor("wk", [C, HID], f32r, kind="ExternalInput").ap()
    wv = nc.dram_tensor("wv", [C, HID], f32r, kind="ExternalInput").ap()
    wg = nc.dram_tensor("wg", [C, HID], f32r, kind="ExternalInput").ap()
    bgh = nc.dram_tensor("bgh", [P, HC], f32, kind="ExternalInput").ap()
    wo = nc.dram_tensor("wo", [HID, C], bf16, kind="ExternalInput").ap()
    bo_bc = nc.dram_tensor("bo_bc", [P, C], f32, kind="ExternalInput").ap()
    out = nc.dram_tensor("out", [NL, SEQ, C], f32, kind="ExternalOutput").ap()
    dbg = {}
    _flags = set(
        f for f in os.environ.get("BASS_DEBUG_OUT", "").split(",") if f
    )
    if "all" in _flags:
        _flags = {"den", "ot", "rbc", "es", "otg", "osb"}
    _shapes = {
        "den": [NL, H, SEQ],
        "ot": [NL, P, HG, SEQ],
        "rbc": [NL, P, HG, SEQ],
        "es": [NL, P, 2, SEQ],
        "otg": [NL, P, HG, SEQ],
        "osb": [NL, P, C],
    }
    for f in _flags:
        dbg[f] = nc.dram_tensor(
            f + "_dbg", _shapes[f], f32, kind="ExternalOutput"
        ).ap()

    Exp = mybir.ActivationFunctionType.Exp
    Tanh = mybir.ActivationFunctionType.Tanh
    MULT = mybir.AluOpType.mult
    ADD = mybir.AluOpType.add

    with _TileContextSplitWaits(nc) as tc:
        with (
            tc.tile_pool(name="const", bufs=1) as const,
            tc.tile_pool(name="dram", bufs=2, space="DRAM") as drp,
        ):
            # --- constants ---------------------------------------------------
            w_sbs = {}
            for name, w_ap in (("wq", wq), ("wk", wk), ("wv", wv), ("wg", wg)):
                w_sbs[name] = const.tile(
                    [P, CC, HID], f32r, tag=f"w_{name}", name=f"w_{name}"
                )
                nc.sync.dma_start(
                    out=w_sbs[name],
                    in_=w_ap.rearrange("(cc p) h -> p cc h", p=P),
                )
            wo_sb = const.tile([P, HC, C], bf16, tag="w_wo")
            nc.sync.dma_start(
                out=wo_sb, in_=wo.rearrange("(hc p) c -> p hc c", p=P)
            )
            bm_sb = const.tile([P, KC, NL], f32, tag="bm")
            nc.sync.dma_start(out=bm_sb, in_=bm)
            bgh_sb = const.tile([P, HC], f32, tag="bgh")
            nc.sync.dma_start(out=bgh_sb, in_=bgh)
            bo_sb = const.tile([P, C], f32, tag="bo")
            nc.sync.dma_start(out=bo_sb, in_=bo_bc)
            ones_w = const.tile([P, 4], bf16, tag="ones_w")
            nc.vector.memset(ones_w, 1.0)
            ones_bc = const.tile([P, CH], bf16, tag="ones_bc")
            nc.vector.memset(ones_bc, 1.0)
            ident_f = const.tile([P, P], f32, tag="ident_f")
            make_identity(nc, ident_f)
            ident_b = const.tile([P, P], bf16, tag="ident_b")
            nc.vector.tensor_copy(ident_b, ident_f)

            # --- main loop ---------------------------------------------------
            with (
                tc.tile_pool(name="xt", bufs=2) as xt,
                tc.tile_pool(name="pj", bufs=1) as pj,
                tc.tile_pool(name="gh", bufs=2) as gh,
                tc.tile_pool(name="vv", bufs=2) as vv,
                tc.tile_pool(name="ee", bufs=4) as ee,
                tc.tile_pool(name="ot", bufs=2) as ot,
                tc.tile_pool(name="dn", bufs=2) as dn,
                tc.tile_pool(name="gp", bufs=2) as gp,
                tc.tile_pool(name="ou", bufs=2) as ou,
                tc.tile_pool(name="psQ", bufs=2, space="PSUM") as psQ,
                tc.tile_pool(name="psO", bufs=1, space="PSUM") as psO,
                tc.tile_pool(name="psD", bufs=1, space="PSUM") as psD,
                tc.tile_pool(name="psA", bufs=2, space="PSUM") as psA,
            ):
                def emit_xt(n):
                    # A: inputs arrive pre-transposed [C, seq], f32r in DRAM.
                    xTs = {}
                    for name, src_ap in (("q", qx), ("k", kx), ("v", vx)):
                        xT = xt.tile([P, CC, SEQ], f32r, tag=f"xt_{name}")
                        nc.sync.dma_start(
                            out=xT,
                            in_=src_ap[n].rearrange("(cc p) s -> p cc s", p=P),
                        )
                        xTs[name] = xT
                    return xTs

                # Row 0's input DMAs go out BEFORE the big bias_pair const
                # load: the SP queue is in-order, so row 0's projections
                # would otherwise wait for the full 4MB bpt transfer.
                xTs0 = emit_xt(0)
                bpt_sb = const.tile([P, H, KC, SEQ], bf16, tag="bpt")
                for h in range(H):
                    nc.sync.dma_start(
                        out=bpt_sb[:, h],
                        in_=bpt[h].rearrange("(kc p) q -> p kc q", p=P),
                    )

                def emit_front(n, xTs=None):
                    if xTs is None:
                        xTs = emit_xt(n)

                    # B: projections (f32r matmuls, bf16 evacuation)
                    qT = pj.tile([P, HC, SEQ], bf16, tag="qT")
                    kT = pj.tile([P, HC, SEQ], bf16, tag="kT")
                    for dst, wname, src in (
                        (qT, "wq", xTs["q"]),
                        (kT, "wk", xTs["k"]),
                    ):
                        for hc in range(HC):
                            pp = psA.tile([P, SEQ], f32, tag="psA")
                            for cc in range(CC):
                                nc.tensor.matmul(
                                    pp,
                                    w_sbs[wname][:, cc, P * hc : P * (hc + 1)],
                                    src[:, cc, :],
                                    start=(cc == 0),
                                    stop=(cc == CC - 1),
                                )
                            nc.vector.tensor_copy(dst[:, hc, :], pp)

                    gth = gh.tile([P, HC, SEQ], bf16, tag="gth")
                    for hc in range(HC):
                        pp = psA.tile([P, SEQ], f32, tag="psA")
                        for cc in range(CC):
                            nc.tensor.matmul(
                                pp,
                                w_sbs["wg"][:, cc, P * hc : P * (hc + 1)],
                                xTs["q"][:, cc, :],
                                start=(cc == 0),
                                stop=(cc == CC - 1),
                            )
                        # sigmoid(x + bg) = 0.5*tanh((x + bg)/2) + 0.5
                        nc.scalar.activation(
                            gth[:, hc, :],
                            pp,
                            Tanh,
                            bias=bgh_sb[:, hc : hc + 1],
                            scale=0.5,
                        )

                    v_sb = vv.tile([P, KC, H, CH], bf16, tag="v")
                    for rc in range(KC):
                        pp = psA.tile([P, SEQ], f32, tag="psA")
                        for cc in range(CC):
                            nc.tensor.matmul(
                                pp[:, 0:HID],
                                xTs["v"][:, cc, P * rc : P * (rc + 1)],
                                w_sbs["wv"][:, cc, :],
                                start=(cc == 0),
                                stop=(cc == CC - 1),
                            )
                        nc.vector.tensor_copy(
                            v_sb[:, rc, :, :],
                            pp[:, 0:HID].rearrange("p (h c) -> p h c", h=H),
                        )

                    # C: attention
                    oT = ot.tile([P, HG, SEQ], bf16, tag="oT")
                    den = dn.tile([H, SEQ], f32, tag="den")
                    for hg in range(HG):
                        Es = {}
                        for pr in range(2):
                            Es[pr] = ee.tile(
                                [P, KC, 2, SEQ], bf16, tag="E", name=f"E_{pr}"
                            )
                            pe_bias = (hg, pr) in PE_PAIRS
                            for kc in range(KC):
                                sp = psQ.tile([P, 2, SEQ], f32, tag="qk", name="qk")
                                for j in range(2):
                                    h2 = 2 * pr + j
                                    nc.tensor.matmul(
                                        sp[:, j, :],
                                        kT[
                                            CH * h2 : CH * (h2 + 1),
                                            hg,
                                            P * kc : P * (kc + 1),
                                        ],
                                        qT[CH * h2 : CH * (h2 + 1), hg, :],
                                        start=True,
                                        stop=not pe_bias,
                                        tile_position=(CH * h2, 0),
                                    )
                                h = 4 * hg + 2 * pr
                                if pe_bias:
                                    # additive bias_pair via identity matmuls
                                    for j in range(2):
                                        nc.tensor.matmul(
                                            sp[:, j, :],
                                            ident_b,
                                            bpt_sb[:, h + j, kc, :],
                                            start=False,
                                            stop=True,
                                        )
                                nc.scalar.activation(
                                    Es[pr][:, kc, :, :],
                                    sp,
                                    Exp,
                                    bias=bm_sb[:, kc, n : n + 1],
                                )
                                if not pe_bias:
                                    eng = (
                                        nc.gpsimd
                                        if (hg, pr, kc) in GPS_TRIPLES
                                        else nc.vector
                                    )
                                    eng.tensor_mul(
                                        Es[pr][:, kc, :, :],
                                        Es[pr][:, kc, :, :],
                                        bpt_sb[:, h : h + 2, kc, :],
                                    )

                        # AV: 4-way column tiling -> packed [4*32, q] output
                        # in one PSUM bank; denominators from a second
                        # column-tiled pass with ones weights.
                        po = psO.tile([P, SEQ], f32, tag="po", name="po")
                        pd = psD.tile([P, SEQ], f32, tag="pd", name="pd")
                        for kc in range(KC):
                            for h2 in range(4):
                                e_rhs = Es[h2 // 2][:, kc, h2 % 2, :]
                                nc.tensor.matmul(
                                    po[CH * h2 : CH * (h2 + 1), :],
                                    v_sb[:, kc, 4 * hg + h2, :],
                                    e_rhs,
                                    start=(kc == 0),
                                    stop=(kc == KC - 1),
                                    tile_position=(0, CH * h2),
                                )
                            for h2 in range(4):
                                e_rhs = Es[h2 // 2][:, kc, h2 % 2, :]
                                nc.tensor.matmul(
                                    pd[CH * h2 : CH * h2 + 1, :],
                                    ones_w[:, h2 : h2 + 1],
                                    e_rhs,
                                    start=(kc == 0),
                                    stop=(kc == KC - 1),
                                    tile_position=(0, CH * h2),
                                )
                        nc.vector.tensor_copy(oT[:, hg, :], po)
                        pds = ot.tile([P, SEQ], f32, tag="pds", name="pds")
                        nc.vector.tensor_copy(pds, pd)
                        nc.sync.dma_start(
                            out=den[4 * hg : 4 * (hg + 1), :],
                            in_=pds[0:P:CH, :],
                        )
                        if "es" in dbg and hg == 0:
                            ef = ot.tile([P, KC, 2, SEQ], f32, tag="ef", name="ef")
                            nc.vector.tensor_copy(ef, Es[0])
                            nc.sync.dma_start(out=dbg["es"][n], in_=ef[:, 0])

                    if "ot" in dbg:
                        otf = ot.tile([P, HG, SEQ], f32, tag="otf", name="otf")
                        nc.vector.tensor_copy(otf, oT)
                        nc.sync.dma_start(out=dbg["ot"][n], in_=otf)
                    if "den" in dbg:
                        nc.sync.dma_start(out=dbg["den"][n], in_=den)

                    return (n, oT, den, gth)

                def emit_tail_head(state):
                    # D1: normalize + gate fold. Emitted right after the
                    # row's front so the serial chain recip -> broadcast ->
                    # fold -> gate overlaps the NEXT row's front entirely.
                    n, oT, den, gth = state
                    rden = dn.tile([H, SEQ], f32, tag="rden")
                    nc.vector.reciprocal(rden, den)
                    dscr = drp.tile([H, SEQ], f32, tag="dscr")
                    nc.sync.dma_start(out=dscr, in_=rden)
                    rbc = gp.tile([P, HG, SEQ], f32, tag="rbc")
                    oTg = gp.tile([P, HG, SEQ], bf16, tag="oTg")
                    for h in range(H):
                        nc.sync.dma_start(
                            out=rbc[CH * (h % 4) : CH * (h % 4 + 1), h // 4, :],
                            in_=dscr[h : h + 1, :].to_broadcast([CH, SEQ]),
                        )
                    for hc in range(HC):
                        nc.gpsimd.tensor_scalar(
                            gth[:, hc, :], gth[:, hc, :], 0.5, 0.5, MULT, ADD
                        )
                        nc.gpsimd.tensor_mul(
                            rbc[:, hc, :], rbc[:, hc, :], gth[:, hc, :]
                        )
                        nc.vector.tensor_mul(
                            oTg[:, hc, :], oT[:, hc, :], rbc[:, hc, :]
                        )
                    if "rbc" in dbg:
                        rbf = ot.tile([P, HG, SEQ], f32, tag="rbf", name="rbf")
                        nc.vector.tensor_copy(rbf, rbc)
                        nc.sync.dma_start(out=dbg["rbc"][n], in_=rbf)
                    if "otg" in dbg:
                        ogf = ot.tile([P, HG, SEQ], f32, tag="ogf", name="ogf")
                        nc.vector.tensor_copy(ogf, oTg)
                        nc.sync.dma_start(out=dbg["otg"][n], in_=ogf)
                    return (n, oTg)

                def emit_tail_tail(state):
                    # D2: output projection, emitted one row late.
                    n, oTg = state
                    for qc in range(QC):
                        pp = psA.tile([P, SEQ], f32, tag="psA")
                        for hc in range(HC):
                            nc.tensor.matmul(
                                pp[:, 0:C],
                                oTg[:, hc, P * qc : P * (qc + 1)],
                                wo_sb[:, hc, :],
                                start=(hc == 0),
                                stop=(hc == HC - 1),
                            )
                        osb = ou.tile([P, C], f32, tag="osb")
                        nc.vector.tensor_add(osb, pp[:, 0:C], bo_sb)
                        if "osb" in dbg and qc == 0:
                            nc.sync.dma_start(out=dbg["osb"][n], in_=osb)
                        nc.sync.dma_start(
                            out=out[n, P * qc : P * (qc + 1), :], in_=osb
                        )

                pending = None
                for n in range(NL):
                    state = emit_front(n, xTs0 if n == 0 else None)
                    head = emit_tail_head(state)
                    if pending is not None:
                        emit_tail_tail(pending)
                    pending = head
                emit_tail_tail(pending)

    return nc


_NC_CACHE = None


def _get_nc():
    global _NC_CACHE
    if _NC_CACHE is None:
        _NC_CACHE = _build_nc()
    return _NC_CACHE


def _to_bf16(a):
    import ml_dtypes

    return np.asarray(a, dtype=ml_dtypes.bfloat16)


def _prepare_in_maps(q_x, k_x, v_x, bias_mask, bias_pair, wq, wk, wv, wg, bg, wo, bo):
    wq_s = np.ascontiguousarray(wq / math.sqrt(CH), dtype=np.float32)
    bpt = np.ascontiguousarray(
        np.transpose(bias_pair[0, 0], (0, 2, 1)), dtype=np.float32
    )  # [h, k, q]
    # heads in PE_PAIRS use the additive PE-identity path: raw bias;
    # the rest are multiplicative: exp(bias)
    _pe_heads = {4 * hg + 2 * pr + j for (hg, pr) in PE_PAIRS for j in range(2)}
    for _h in range(H):
        if _h not in _pe_heads:
            bpt[_h] = np.exp(bpt[_h])
    bpt_exp = _to_bf16(bpt)
    bgh = np.ascontiguousarray((bg / 2.0).reshape(HC, P).T, dtype=np.float32)
    bo_bc = np.ascontiguousarray(np.tile(bo[None, :], (P, 1)), dtype=np.float32)
    bm_all = np.asarray(bias_mask[0, :, 0, 0, :], dtype=np.float32)  # [64, 512]
    wo_b = _to_bf16(wo)

    in_maps = []
    for c in range(N_CORES):
        ns = slice(NL * c, NL * (c + 1))
        bm_r = np.ascontiguousarray(
            bm_all[ns].reshape(NL, KC, P).transpose(2, 1, 0), dtype=np.float32
        )
        in_maps.append(
            {
                "qx": np.ascontiguousarray(
                    q_x[0, ns].transpose(0, 2, 1), dtype=np.float32
                ),
                "kx": np.ascontiguousarray(
                    k_x[0, ns].transpose(0, 2, 1), dtype=np.float32
                ),
                "vx": np.ascontiguousarray(
                    v_x[0, ns].transpose(0, 2, 1), dtype=np.float32
                ),
                "bpt": bpt_exp,
                "bm": bm_r,
                "wq": wq_s,
                "wk": np.ascontiguousarray(wk, dtype=np.float32),
                "wv": np.ascontiguousarray(wv, dtype=np.float32),
                "wg": np.ascontiguousarray(wg, dtype=np.float32),
                "bgh": bgh,
                "wo": wo_b,
                "bo_bc": bo_bc,
            }
        )
    return in_maps


def run(trace=False, **inputs):
    """Run the kernel; returns (output, BassKernelResults)."""
    args = {k: np.asarray(v) for k, v in inputs.items()}
    in_maps = _prepare_in_maps(
        args["q_x"], args["k_x"], args["v_x"], args["bias_mask"],
        args["bias_pair"], args["wq"], args["wk"], args["wv"], args["wg"],
        args["bg"], args["wo"], args["bo"],
    )
    nc = _get_nc()
    res = run_bass_kernel_spmd(nc, in_maps, list(range(N_CORES)), trace=trace)
    out = np.empty((1, NL * N_CORES, SEQ, C), dtype=np.float32)
    for c in range(N_CORES):
        out[0, NL * c : NL * (c + 1)] = res.results[c]["out"]
    return out, res


def kernel(**inputs):
    out, _ = run(trace=False, **inputs)
    return out


if __name__ == "__main__":
    rng = np.random.default_rng(0)
    demo = {
        "q_x": rng.standard_normal((1, 64, SEQ, C)).astype(np.float32),
        "k_x": rng.standard_normal((1, 64, SEQ, C)).astype(np.float32),
        "v_x": rng.standard_normal((1, 64, SEQ, C)).astype(np.float32),
        "bias_mask": rng.standard_normal((1, 64, 1, 1, SEQ)).astype(np.float32),
        "bias_pair": rng.standard_normal((1, 1, H, SEQ, SEQ)).astype(np.float32),
        "wq": (rng.standard_normal((C, HID)) / 16).astype(np.float32),
        "wk": (rng.standard_normal((C, HID)) / 16).astype(np.float32),
        "wv": (rng.standard_normal((C, HID)) / 16).astype(np.float32),
        "wg": (rng.standard_normal((C, HID)) * 0.02).astype(np.float32),
        "bg": np.ones((HID,), dtype=np.float32),
        "wo": (rng.standard_normal((HID, C)) * 0.02).astype(np.float32),
        "bo": np.zeros((C,), dtype=np.float32),
    }
    o = kernel(**demo)
    print("kernel ran, out shape", o.shape, "mean", float(np.abs(o).mean()))
